# revision 68
# baseline (speedup 1.0000x reference)
"""Trainium2 Bass kernel for nn_Attention_45749991637079.

Reference computation (per batch b, C=192 channels, 128x128 image, 8 heads):
    qkv  = w_qkv @ x                       (1x1 conv; k-branch unused)
    q,v  = depthwise 3x3 (SAME) of the q/v channel blocks
    qd   = q[:, ::2, ::2]                  (64x64 downsample)
    attn = softmax(l2norm-rows(qd_h) gram * temp)   per head (24x24)
    out  = w_proj @ (attn @ v)             == (w_proj @ blockdiag(attn)) @ v

Sharding: data-parallel over batch; one batch per NeuronCore (8 cores).

Device algorithm per core (optimized for a warm, densely-fed PE):
  Q phase: whole-image pointwise conv into a zero-padded SBUF buffer
    (130x130 per chunk), 9-tap depthwise via diag-matmuls with stride-2
    views -> qd; per-128-col-block PE transposes -> gram accumulate.
  Softmax: row-norm scaling, blockwise softmax over all 24-col blocks,
    block-diagonal mask, Wf = blockdiag(A)^T-contraction with WpT.
  V phase (overlaps softmax): pointwise -> padded buffer -> 9-tap
    depthwise -> fp16 stage -> fused final matmul with Wf -> DMA out.
  All weight transposes / diag-tap matrices / masks precomputed on host.
"""

import numpy as np

C = 192
H = W = 128
HW = H * W
HEADS = 8
CHD = 24
P0, P1 = 128, 64          # channel partition chunks: 0:128 and 128:192
PB = 130                  # padded row width / height (1 + 128 + 1)
PBSZ = PB * PB            # padded image cols per chunk
SUB = 512                 # output subtile cols (4 image rows)
NSUB = HW // SUB          # 32
CHUNK = 2048              # x DMA chunk cols (16 image rows)
NCH = HW // CHUNK         # 8
QSUB = 8                  # qd subtiles (8 rows of 64 each)
TAPS = [(di, dj) for di in range(3) for dj in range(3)]

_BUILT = {}


def _build(iters=1):
    import concourse.mybir as mybir
    import concourse.tile as tile
    from concourse import bacc

    f32 = mybir.dt.float32
    f16 = mybir.dt.float16
    Alu = mybir.AluOpType
    Act = mybir.ActivationFunctionType
    Ax = mybir.AxisListType

    nc = bacc.Bacc(
        "TRN2", target_bir_lowering=False, debug=False,
        enable_asserts=False, num_devices=8,
    )

    # DRAM I/O (per-core shapes)
    xb = nc.dram_tensor("xb", (C, HW), f16, kind="ExternalInput").ap()
    wq = nc.dram_tensor("wq", (P0, 512), f16, kind="ExternalInput").ap()
    wv = nc.dram_tensor("wv", (P0, 512), f16, kind="ExternalInput").ap()
    wp = nc.dram_tensor("wp", (P0, 384), f16, kind="ExternalInput").ap()
    dq = nc.dram_tensor("dq", (P0, 9 * P0), f16, kind="ExternalInput").ap()
    dq1 = nc.dram_tensor("dq1", (P0, 6 * P0), f16, kind="ExternalInput").ap()
    dv = nc.dram_tensor("dv", (P0, 9 * P0), f16, kind="ExternalInput").ap()
    dv1 = nc.dram_tensor("dv1", (P0, 6 * P0), f16, kind="ExternalInput").ap()
    tq = nc.dram_tensor("tq", (C, 1), f32, kind="ExternalInput").ap()
    eye = nc.dram_tensor("eye", (P0, P0), f16, kind="ExternalInput").ap()
    mk0 = nc.dram_tensor("mk0", (P0, C), f32, kind="ExternalInput").ap()
    mk1 = nc.dram_tensor("mk1", (P1, C), f32, kind="ExternalInput").ap()
    out = nc.dram_tensor("out", (C, HW), f32, kind="ExternalOutput").ap()
    import os
    _dbg = os.environ.get("KDBG") == "1"
    if _dbg:
        dqd = nc.dram_tensor("dqd", (P0, 8192), f16, kind="ExternalOutput").ap()
        datt0 = nc.dram_tensor("datt0", (P0, C), f32, kind="ExternalOutput").ap()
        datt1 = nc.dram_tensor("datt1", (P1, C), f32, kind="ExternalOutput").ap()
        dwf = nc.dram_tensor("dwf", (P0, 512), f16, kind="ExternalOutput").ap()
        dgram = nc.dram_tensor("dgram", (P0, C), f32, kind="ExternalOutput").ap()
        dpb = nc.dram_tensor("dpb", (P0, 2 * PBSZ), f16, kind="ExternalOutput").ap()

    import contextlib

    with tile.TileContext(nc) as tc:
      with (tc.For_i(0, iters, 1) if iters > 1 else contextlib.nullcontext()):
        with (
            tc.tile_pool(name="const", bufs=1) as cp,
            tc.tile_pool(name="work", bufs=5) as wkp,
            tc.tile_pool(name="vst", bufs=12) as vsp,
            tc.tile_pool(name="ost", bufs=2) as osp,
            tc.tile_pool(name="psPW", bufs=3, space="PSUM") as psPW,
            tc.tile_pool(name="psT0", bufs=3, space="PSUM") as psT0,
            tc.tile_pool(name="psT1", bufs=2, space="PSUM") as psT1,
        ):
            # ---- constants ----
            # pw weights packed channel-padded to 256: [:,0:256]=WT_pad rows
            # 0:128 (K-chunk0), [:,256:512]=rows 128:256 (K-chunk1, rows 64:128
            # zero).  Uniform [128,128] lhsT slices keep the PE weight loads
            # pipelined (mixed tile shapes serialize LDWEIGHTS).
            wq_sb = cp.tile([P0, 512], f16)
            wv_sb = cp.tile([P0, 512], f16)
            wp_sb = cp.tile([P0, 384], f16)   # WpT rows 0:128 | rows 128:192
            dq_sb = cp.tile([P0, 9 * P0], f16)
            dq1_sb = cp.tile([P0, 6 * P0], f16)  # ch1 paired-tap matrices
            dv_sb = cp.tile([P0, 9 * P0], f16)
            dv1_sb = cp.tile([P0, 6 * P0], f16)
            tq_sb = cp.tile([P0, 2], f32)     # [:,0]=ch0..127, [0:64,1]=ch128..191
            eye_sb = cp.tile([P0, P0], f16)
            mk0_sb = cp.tile([P0, C], f32)    # blockdiag mask rows 0:128
            mk1_sb = cp.tile([P1, C], f32)    # rows 128:192
            pb = cp.tile([P0, 2 * PBSZ], f16)  # padded image: ch0 | ch1
            qd_sb = cp.tile([P0, 8192], f16)  # qd: [:,0:4096] | [0:64,4096:8192]
            g0a = cp.tile([P0, C], f32)       # gram accumulator rows 0:128
            g1a = cp.tile([P1, C], f32)       # rows 128:192
            srow = cp.tile([P0, C], f32)      # s_d broadcast to all partitions
            wf_sb = cp.tile([P0, 512], f16)   # WfT M-padded: K0 @0:256, K1 @256:512
            A0h = cp.tile([P0, C], f16)       # masked softmax attn (fp16)
            A1h = cp.tile([P1, C], f16)
            ssq = cp.tile([P0, 2 * QSUB], f32)  # row sum-of-squares per subtile
            stats = cp.tile([P0, 4 * HEADS], f32)  # softmax stats
            rn = cp.tile([P0, 2], f32)        # 1/||q|| * sqrt(temp)
            scr = cp.tile([P0, SUB], f32)     # scratch for sumsq STT

            pbv0 = pb[:, 0:PBSZ].rearrange("p (r c) -> p r c", c=PB)
            pbv1 = pb[0:P1, PBSZ:2 * PBSZ].rearrange("p (r c) -> p r c", c=PB)
            pbf1 = pb[:, PBSZ:2 * PBSZ].rearrange("p (r c) -> p r c", c=PB)

            # manual 3-slot x ring: ch1 junk partitions zeroed once (on DVE,
            # which is idle at the start) so the channel-padded pw matmuls
            # never stream NaN bit patterns.
            xslots = []
            for _i in range(3):
                xs_t = cp.tile([P0, 2 * CHUNK], f16, name=f"xslot{_i}")
                xslots.append(xs_t)
            for xs in xslots:
                nc.vector.memset(xs[P1:P0, CHUNK:2 * CHUNK], 0.0)

            def dma_xchunk(c):
                xt = xslots[c % 3]
                cs = slice(c * CHUNK, (c + 1) * CHUNK)
                nc.sync.dma_start(out=xt[:, 0:CHUNK], in_=xb[0:P0, cs])
                nc.sync.dma_start(out=xt[0:P1, CHUNK:2 * CHUNK], in_=xb[P0:C, cs])
                return xt

            # HAM warm-up: dependency-free matmuls into dead PSUM slots keep
            # the PE busy through its 3.4us activity window while the first
            # DMAs land, so the real stream starts at full clock.
            for wu in range(16):
                pwu = (psT0 if wu % 2 == 0 else psT1).tile(
                    [P0, 256], f32, tag="t", name="warm")
                nc.tensor.matmul(pwu[:], pb[:, 0:P0], pb[:, 1024:1280],
                                 start=True, stop=True)

            # the first pw matmuls need only wq + x chunk 0, and each
            # dma_start costs ~600ns of Sync issue time: those go first.
            nc.sync.dma_start(out=wq_sb[:], in_=wq[:])
            xq = [dma_xchunk(0)]
            xq.append(dma_xchunk(1))
            nc.sync.dma_start(out=dq_sb[:], in_=dq[:])
            nc.sync.dma_start(out=dq1_sb[:], in_=dq1[:])
            nc.sync.dma_start(out=wv_sb[:], in_=wv[:])
            nc.sync.dma_start(out=wp_sb[:, 0:192], in_=wp[:, 0:192])
            nc.sync.dma_start(out=wp_sb[0:P1, 192:384], in_=wp[0:P1, 192:384])
            nc.sync.dma_start(out=dv_sb[:], in_=dv[:])
            nc.sync.dma_start(out=dv1_sb[:], in_=dv1[:])
            nc.sync.dma_start(out=tq_sb[:, 0:1], in_=tq[0:P0, :])
            nc.sync.dma_start(out=tq_sb[0:P1, 1:2], in_=tq[P0:C, :])
            nc.sync.dma_start(out=eye_sb[:], in_=eye[:])
            nc.sync.dma_start(out=mk0_sb[:], in_=mk0[:])
            nc.sync.dma_start(out=mk1_sb[:], in_=mk1[:])

            # zero the pad rows/cols once; the ch1 region is padded across
            # all 128 partitions (its interior rows 64:128 are rewritten with
            # real zeros by every full-partition pw copy).
            nc.gpsimd.memset(pbv0[:, :, 0:1], 0.0)
            nc.gpsimd.memset(pbv0[:, :, PB - 1:PB], 0.0)
            nc.gpsimd.memset(pbv0[:, 0, :], 0.0)
            nc.gpsimd.memset(pbv0[:, PB - 1, :], 0.0)
            nc.gpsimd.memset(pbf1[:, :, 0:1], 0.0)
            nc.gpsimd.memset(pbf1[:, :, PB - 1:PB], 0.0)
            nc.gpsimd.memset(pbf1[:, 0, :], 0.0)
            nc.gpsimd.memset(pbf1[:, PB - 1, :], 0.0)
            # upper duplicate stores image col m at buffer col m; its cols
            # 128..129 are never written and must stay zero
            nc.gpsimd.memset(pbf1[P1:P0, :, P0:PB - 1], 0.0)

            nc.gpsimd.memset(g0a[:], 0.0)
            nc.gpsimd.memset(g1a[:], 0.0)
            nc.gpsimd.memset(wf_sb[:], 0.0)   # zero rows/cols of the padding

            def pw_subtile(s, xt, s4, w_sb, act_first):
                """Pointwise conv of image rows 4s..4s+3 into padded buffer.
                All 4 matmuls use uniform [128,128] lhsT (channel-padded);
                junk rows 64:128 of the ch1 rhs meet zero weight rows."""
                xr0 = xt[:, s4 * SUB:(s4 + 1) * SUB]
                xr1 = xt[:, CHUNK + s4 * SUB:CHUNK + (s4 + 1) * SUB]
                pp0 = psPW.tile([P0, SUB], f32, tag="pw")
                pp1 = psPW.tile([P0, SUB], f32, tag="pw")
                nc.tensor.matmul(pp0[:], w_sb[:, 0:128], xr0, start=True, stop=False)
                nc.tensor.matmul(pp1[:], w_sb[:, 128:256], xr0, start=True, stop=False)
                nc.tensor.matmul(pp0[:], w_sb[:, 256:384], xr1, start=False, stop=True)
                nc.tensor.matmul(pp1[:], w_sb[:, 384:512], xr1, start=False, stop=True)
                r0 = 4 * s + 1   # buffer row of image row 4s
                d0 = pbv0[:, r0:r0 + 4, 1:129]
                # ch1 lands twice: lower copy at the standard +1 offset and
                # the duplicated upper partitions shifted one column left,
                # which lets two horizontal taps share one matmul.
                d1a = pbf1[0:P1, r0:r0 + 4, 1:129]
                d1b = pbf1[P1:P0, r0:r0 + 4, 0:128]
                v0 = pp0[:].rearrange("p (r c) -> p r c", c=W)
                v1 = pp1[:].rearrange("p (r c) -> p r c", c=W)
                if act_first:
                    nc.scalar.copy(d0, v0)
                    nc.vector.tensor_copy(d1a, v1[0:P1])
                    nc.vector.tensor_copy(d1b, v1[P1:P0])
                else:
                    nc.vector.tensor_copy(d0, v0)
                    nc.scalar.copy(d1a, v1[0:P1])
                    nc.scalar.copy(d1b, v1[P1:P0])

            # ================= Q phase =================
            # pointwise runs one chunk ahead of the taps, and the gram of
            # chunk c runs one iteration late so its transposes never wait
            # on the freshly-written qd copies.
            for s4 in range(4):
                pw_subtile(s4, xq[0], s4, wq_sb, s4 % 2 == 0)

            def emit_gram(k):
                """Gram contribution of qd subtile k's 4 128-col blocks
                (matmuls batched by lhsT shape to keep weight loads
                pipelined)."""
                pg0 = psT0.tile([P0, C], f32, tag="t")
                pg1 = psT1.tile([P1, C], f32, tag="t")
                qdTs = []
                for b4 in range(4):
                    kcol = k * SUB + b4 * P0
                    pt0 = psPW.tile([P0, P0], f16, tag="pw")
                    pt1 = psPW.tile([P0, P1], f16, tag="pw")
                    nc.tensor.transpose(pt0[:], qd_sb[:, kcol:kcol + P0], eye_sb[:])
                    nc.tensor.transpose(pt1[:], qd_sb[0:P1, 4096 + kcol:4096 + kcol + P0],
                                        eye_sb[0:P1, 0:P1])
                    qdT = wkp.tile([P0, C], f16, tag="qdT")
                    nc.vector.tensor_copy(qdT[:, 0:P0], pt0[:])
                    nc.vector.tensor_copy(qdT[:, P0:C], pt1[:])
                    qdTs.append(qdT)
                # the masked softmax only reads the diagonal head blocks:
                # rows 0:128 need cols 0:144 (heads 0-5), rows 128:192 need
                # cols 120:192 (heads 5-7); the rest of g0a/g1a stays zero.
                for b4 in range(4):
                    nc.tensor.matmul(pg0[:, 0:144], qdTs[b4][:, 0:P0],
                                     qdTs[b4][:, 0:144],
                                     start=(b4 == 0), stop=(b4 == 3))
                for b4 in range(4):
                    nc.tensor.matmul(pg1[:, 0:72], qdTs[b4][:, P0:C],
                                     qdTs[b4][:, 120:192],
                                     start=(b4 == 0), stop=(b4 == 3))
                nc.vector.tensor_tensor(g0a[:, 0:144], g0a[:, 0:144],
                                        pg0[:, 0:144], Alu.add)
                nc.vector.tensor_tensor(g1a[:, 120:192], g1a[:, 120:192],
                                        pg1[:, 0:72], Alu.add)

            for c in range(NCH):
                if c + 1 < NCH:
                    for s4 in range(4):
                        pw_subtile(4 * (c + 1) + s4, xq[c + 1], s4, wq_sb,
                                   s4 % 2 == 0)
                    if c + 2 < NCH:
                        xq.append(dma_xchunk(c + 2))
                # taps for qd subtile k=c (qd rows 8c..8c+7); ch1 uses the
                # zero-padded [128,128] diagonals on the full-partition view
                k = c
                pq0 = psT0.tile([P0, SUB], f32, tag="t")
                pq1 = psT1.tile([P0, SUB], f32, tag="t")
                o0 = pq0[:].rearrange("p (r c) -> p r c", c=64)
                o1 = pq1[:].rearrange("p (r c) -> p r c", c=64)
                rb = 16 * k  # buffer row of qd row 8k input base (2i, i=8k)
                for t, (di, dj) in enumerate(TAPS):
                    rhs0 = pbv0[:, rb + di:rb + di + 16:2, dj:dj + 128:2]
                    nc.tensor.matmul(o0, dq_sb[:, t * P0:(t + 1) * P0], rhs0,
                                     start=(t == 0), stop=(t == 8))
                for g in range(6):
                    di, dj = (g, 0) if g < 3 else (g - 3, 2)
                    rhs1 = pbf1[:, rb + di:rb + di + 16:2, dj:dj + 128:2]
                    nc.tensor.matmul(o1, dq1_sb[:, g * P0:(g + 1) * P0], rhs1,
                                     start=(g == 0), stop=(g == 5))
                nc.scalar.activation(scr[:], pq0[:], Act.Square,
                                     accum_out=ssq[:, k:k + 1])
                nc.scalar.activation(scr[0:P1, :], pq1[0:P1, :], Act.Square,
                                     accum_out=ssq[0:P1, QSUB + k:QSUB + k + 1])
                nc.vector.tensor_copy(qd_sb[:, k * SUB:(k + 1) * SUB], pq0[:])
                nc.vector.tensor_copy(qd_sb[0:P1, 4096 + k * SUB:4096 + (k + 1) * SUB],
                                      pq1[0:P1, :])
                if c >= 1:
                    emit_gram(c - 1)
            emit_gram(NCH - 1)

            if _dbg:
                nc.sync.dma_start(out=dqd, in_=qd_sb[:])
                nc.sync.dma_start(out=dgram, in_=g0a[:])
                nc.sync.dma_start(out=dpb, in_=pb[:])
            # ================= V phase helpers =================
            # taps for subtile j need pad rows 4j..4j+5; row 4j+5 is written
            # by pw subtile j+1, so taps lag the pointwise by one subtile.
            def emit_taps_v(j):
                pv0 = psT0.tile([P0, SUB], f32, tag="t")
                pv1 = psT1.tile([P0, SUB], f32, tag="t")
                o0 = pv0[:].rearrange("p (r c) -> p r c", c=W)
                o1 = pv1[:].rearrange("p (r c) -> p r c", c=W)
                rb = 4 * j
                for t, (di, dj) in enumerate(TAPS):
                    rhs0 = pbv0[:, rb + di:rb + di + 4, dj:dj + 128]
                    nc.tensor.matmul(o0, dv_sb[:, t * P0:(t + 1) * P0], rhs0,
                                     start=(t == 0), stop=(t == 8))
                for g in range(6):
                    di, dj = (g, 0) if g < 3 else (g - 3, 2)
                    rhs1 = pbf1[:, rb + di:rb + di + 4, dj:dj + 128]
                    nc.tensor.matmul(o1, dv1_sb[:, g * P0:(g + 1) * P0], rhs1,
                                     start=(g == 0), stop=(g == 5))
                vst0 = vsp.tile([P0, SUB], f16, tag="v0")
                vst1 = vsp.tile([P0, SUB], f16, tag="v1")
                nc.vector.tensor_copy(vst0[:], pv0[:])
                nc.vector.tensor_copy(vst1[:], pv1[:])
                return vst0, vst1

            och = {}

            def emit_final(j, vst0, vst1):
                """Fused final matmul: out = WfT-contraction @ v_dw.  All 4
                lhsT slices are uniform [128,128]; vst1 junk rows 64:128 meet
                the zero rows of the padded Wf."""
                c, s4 = j // 4, j % 4
                if s4 == 0:
                    och[c] = (osp.tile([P0, CHUNK], f32, tag="o0", name="och0"),
                              osp.tile([P1, CHUNK], f32, tag="o1", name="och1"))
                och0, och1 = och[c]
                po0 = psT0.tile([P0, SUB], f32, tag="t")
                po1 = psT1.tile([P0, SUB], f32, tag="t")
                nc.tensor.matmul(po0[:], wf_sb[:, 0:128], vst0[:],
                                 start=True, stop=False)
                nc.tensor.matmul(po1[:], wf_sb[:, 128:256], vst0[:],
                                 start=True, stop=False)
                nc.tensor.matmul(po0[:], wf_sb[:, 256:384], vst1[:],
                                 start=False, stop=True)
                nc.tensor.matmul(po1[:], wf_sb[:, 384:512], vst1[:],
                                 start=False, stop=True)
                nc.scalar.copy(och0[:, s4 * SUB:(s4 + 1) * SUB], po0[:])
                nc.vector.tensor_copy(och1[:, s4 * SUB:(s4 + 1) * SUB],
                                      po1[0:P1, :])
                if c == NCH - 1:
                    # last chunk: flush per subtile so the tail DMA is short
                    scs = slice(c * CHUNK + s4 * SUB, c * CHUNK + (s4 + 1) * SUB)
                    ssl = slice(s4 * SUB, (s4 + 1) * SUB)
                    nc.sync.dma_start(out=out[0:P0, scs], in_=och0[:, ssl])
                    nc.sync.dma_start(out=out[P0:C, scs], in_=och1[:, ssl])
                elif s4 == 3:
                    ocs = slice(c * CHUNK, (c + 1) * CHUNK)
                    nc.sync.dma_start(out=out[0:P0, ocs], in_=och0[:])
                    nc.sync.dma_start(out=out[P0:C, ocs], in_=och1[:])

            # Pre-emit the first two V chunks' pointwise and six tap groups
            # so the in-order PE queue has work while the softmax chain runs.
            xv = [dma_xchunk(0)]
            for s4 in range(4):
                pw_subtile(s4, xv[0], s4, wv_sb, s4 % 2 == 0)
            xv.append(dma_xchunk(1))
            xv.append(dma_xchunk(2))
            for s4 in range(4):
                pw_subtile(4 + s4, xv[1], s4, wv_sb, s4 % 2 == 0)
            xv.append(dma_xchunk(3))
            pend = [(j,) + emit_taps_v(j) for j in range(6)]

            # ---- row scales: rn = sqrt(temp) / ||qd_row|| ----
            # ACT Sqrt is low-precision; one Newton step on y=sqrt(ss).
            nc.vector.tensor_reduce(ssq[:, 0:1], ssq[:, 0:QSUB], Ax.X, Alu.add)
            nc.vector.tensor_reduce(ssq[0:P1, QSUB:QSUB + 1],
                                    ssq[0:P1, QSUB:2 * QSUB], Ax.X, Alu.add)
            for ss_ap, rn_ap, tq_ap in (
                (ssq[:, 0:1], rn[:, 0:1], tq_sb[:, 0:1]),
                (ssq[0:P1, QSUB:QSUB + 1], rn[0:P1, 1:2], tq_sb[0:P1, 1:2]),
            ):
                y = scr[0:ss_ap.shape[0], 0:1]
                yr = scr[0:ss_ap.shape[0], 1:2]
                nc.scalar.activation(y, ss_ap, Act.Sqrt)
                nc.vector.reciprocal(yr, y)                      # 1/y
                nc.vector.tensor_tensor(yr, yr, ss_ap, Alu.mult)  # ss/y
                nc.vector.tensor_tensor(y, y, yr, Alu.add)
                nc.vector.tensor_scalar_mul(y, y, 0.5)            # refined sqrt
                nc.vector.reciprocal(rn_ap, y)
                nc.vector.tensor_tensor(rn_ap, rn_ap, tq_ap, Alu.mult)

            # attn = diag(s) G diag(s): row scale by s_c, then elementwise
            # multiply by s_d replicated across partitions.
            nc.sync.dma_start(out=srow[0:1, 0:P0], in_=rn[:, 0:1])
            nc.sync.dma_start(out=srow[0:1, P0:C], in_=rn[0:P1, 1:2])
            nc.gpsimd.partition_broadcast(srow[:], srow[0:1, :])
            nc.vector.tensor_scalar_mul(g0a[:], g0a[:], rn[:, 0:1])
            nc.vector.tensor_scalar_mul(g1a[:], g1a[:], rn[0:P1, 1:2])
            nc.vector.tensor_tensor(g0a[:], g0a[:], srow[:], Alu.mult)
            nc.vector.tensor_tensor(g1a[:], g1a[:], srow[0:P1, :], Alu.mult)

            # ---- blockwise softmax over every 24-col block, then keep the
            # diagonal block per row via a precomputed mask ----
            gv0 = g0a[:].rearrange("p (h c) -> p h c", c=CHD)
            gv1 = g1a[:].rearrange("p (h c) -> p h c", c=CHD)
            # logits are cosine-similarities scaled by temperature (=1):
            # bounded, so exp is safe without the max-subtraction pass
            nc.scalar.activation(g0a[:], g0a[:], Act.Exp)
            nc.scalar.activation(g1a[:], g1a[:], Act.Exp)
            sm0 = stats[:, 2 * HEADS:3 * HEADS]
            sm1 = stats[0:P1, 3 * HEADS:4 * HEADS]
            nc.vector.tensor_reduce(sm0, gv0, Ax.X, Alu.add)
            nc.vector.tensor_reduce(sm1, gv1, Ax.X, Alu.add)
            nc.vector.reciprocal(sm0, sm0)
            nc.vector.reciprocal(sm1, sm1)
            nc.vector.tensor_tensor(gv0, gv0,
                                    sm0.unsqueeze(2).broadcast_to((P0, HEADS, CHD)),
                                    Alu.mult)
            nc.vector.tensor_tensor(gv1, gv1,
                                    sm1.unsqueeze(2).broadcast_to((P1, HEADS, CHD)),
                                    Alu.mult)
            nc.vector.tensor_tensor(g0a[:], g0a[:], mk0_sb[:], Alu.mult)
            nc.vector.tensor_tensor(g1a[:], g1a[:], mk1_sb[:], Alu.mult)
            nc.vector.tensor_copy(A0h[:], g0a[:])
            nc.vector.tensor_copy(A1h[:], g1a[:])
            if _dbg:
                nc.sync.dma_start(out=datt0, in_=g0a[:])
                nc.sync.dma_start(out=datt1, in_=g1a[:])

            # ---- WfT = blockdiag(A)-contraction with WpT (fp16) ----
            pwf0 = psT0.tile([P0, C], f32, tag="t")
            pwf1 = psT1.tile([P1, C], f32, tag="t")
            nc.tensor.matmul(pwf0[:], A0h[:, 0:P0], wp_sb[:, 0:192],
                             start=True, stop=False)
            nc.tensor.matmul(pwf0[:], A1h[:, 0:P0], wp_sb[0:P1, 192:384],
                             start=False, stop=True)
            nc.tensor.matmul(pwf1[:], A0h[:, P0:C], wp_sb[:, 0:192],
                             start=True, stop=False)
            nc.tensor.matmul(pwf1[:], A1h[:, P0:C], wp_sb[0:P1, 192:384],
                             start=False, stop=True)
            nc.scalar.copy(wf_sb[:, 0:192], pwf0[:])
            nc.scalar.copy(wf_sb[0:P1, 256:448], pwf1[:])
            if _dbg:
                nc.sync.dma_start(out=dwf, in_=wf_sb[:])

            # ================= V phase (remainder) =================
            for jv, v0p, v1p in pend:
                emit_final(jv, v0p, v1p)
            for s in range(7, NSUB + 1):
                if 8 <= s < NSUB:
                    pw_subtile(s, xv[s // 4], s % 4, wv_sb, s % 2 == 0)
                    if s % 4 == 3 and s // 4 + 2 < NCH:
                        xv.append(dma_xchunk(s // 4 + 2))
                j = s - 1
                v0p, v1p = emit_taps_v(j)
                emit_final(j, v0p, v1p)

    nc.compile()
    return nc


def _host_inputs(x, w_qkv, w_dw, w_proj, temperature):
    """Per-core input maps (host-side precompute of all weight transforms)."""
    f = np.float32
    W_q = w_qkv[0:C].astype(f)           # (192,192)
    W_v = w_qkv[2 * C:3 * C].astype(f)
    wq_d = w_dw[0:C, 0].reshape(C, 9).astype(f)        # (192,9) taps (di,dj)
    wv_d = w_dw[2 * C:3 * C, 0].reshape(C, 9).astype(f)

    def pack_T(Wm):
        """W^T channel-padded to 256 and packed as (128, 512):
        [:,0:256]=rows 0:128 of WT_pad (K-chunk0), [:,256:512]=rows 128:256
        (K-chunk1; rows 64:128 zero).  Output channels 192:256 duplicate
        128:192 so the ch1 image lands twice in the padded buffer, enabling
        paired depthwise taps."""
        WTp = np.zeros((256, 256), f)
        WTp[0:C, 0:C] = Wm.T.astype(f)
        WTp[:, 192:256] = WTp[:, 128:192]
        return np.concatenate([WTp[0:P0], WTp[P0:256]], axis=1)

    def pack_diag(wd, lo, n):
        """9 diagonal tap matrices, zero-padded to (128, 9*128)."""
        out = np.zeros((P0, 9 * P0), f)
        for t in range(9):
            np.fill_diagonal(out[0:n, t * P0:t * P0 + n], wd[lo:lo + n, t])
        return out

    def pack_diag1(wd):
        """ch1 tap matrices for the duplicated layout, (128, 6*128):
        g<3: paired taps (di,dj=0)+(di,dj=1) -- rows 0:64 scale the lower
        copy, rows 64:128 the col-shifted upper copy, both into out 0:64;
        g>=3: single tap (di,dj=2) on the lower copy only."""
        out = np.zeros((P0, 6 * P0), f)
        w = wd[P0:C].reshape(P1, 3, 3)
        for di in range(3):
            np.fill_diagonal(out[0:P1, di * P0:di * P0 + P1], w[:, di, 0])
            np.fill_diagonal(out[P1:P0, di * P0:di * P0 + P1], w[:, di, 1])
            g = 3 + di
            np.fill_diagonal(out[0:P1, g * P0:g * P0 + P1], w[:, di, 2])
        return out

    wp_pack = np.zeros((P0, 384), f)
    WpT = w_proj.T.astype(f)
    wp_pack[:, 0:192] = WpT[0:P0]
    wp_pack[0:P1, 192:384] = WpT[P0:C]

    tq = np.sqrt(np.repeat(temperature.reshape(HEADS).astype(f), CHD)).reshape(C, 1)
    eye = np.eye(P0, dtype=np.float16)

    heads = np.arange(C) // CHD
    mask_full = (heads[:, None] == heads[None, :]).astype(f)   # (192,192)

    shared = {
        "wq": pack_T(W_q), "wv": pack_T(W_v), "wp": wp_pack,
        "dq": pack_diag(wq_d, 0, P0), "dq1": pack_diag1(wq_d),
        "dv": pack_diag(wv_d, 0, P0), "dv1": pack_diag1(wv_d),
        "tq": tq, "eye": eye.astype(np.float16),
        "mk0": np.ascontiguousarray(mask_full[0:P0]),
        "mk1": np.ascontiguousarray(mask_full[P0:C]),
    }
    h = np.float16
    for k in ("wq", "wv", "wp", "dq", "dq1", "dv", "dv1"):
        shared[k] = shared[k].astype(h)
    maps = []
    for b in range(8):
        m = dict(shared)
        m["xb"] = np.ascontiguousarray(x[b].reshape(C, HW).astype(h))
        maps.append(m)
    return maps


def kernel(x, w_qkv, w_dw, w_proj, temperature, _trace=False, _iters=1):
    from concourse.bass_utils import run_bass_kernel_spmd
    if _iters not in _BUILT:
        _BUILT[_iters] = _build(_iters)
    nc = _BUILT[_iters]
    in_maps = _host_inputs(
        np.asarray(x), np.asarray(w_qkv), np.asarray(w_dw),
        np.asarray(w_proj), np.asarray(temperature))
    res = run_bass_kernel_spmd(nc, in_maps, list(range(8)), trace=_trace)
    outs = [res.results[i]["out"].reshape(C, H, W) for i in range(8)]
    y = np.stack(outs, axis=0).astype(np.float32)
    kernel.last_result = res
    return y


# revision 69
# speedup vs baseline: 1.0403x; 1.0403x over previous
"""Trainium2 Bass kernel for nn_Attention_45749991637079.

Reference computation (per batch b, C=192 channels, 128x128 image, 8 heads):
    qkv  = w_qkv @ x                       (1x1 conv; k-branch unused)
    q,v  = depthwise 3x3 (SAME) of the q/v channel blocks
    qd   = q[:, ::2, ::2]                  (64x64 downsample)
    attn = softmax(l2norm-rows(qd_h) gram * temp)   per head (24x24)
    out  = w_proj @ (attn @ v)             == (w_proj @ blockdiag(attn)) @ v

Sharding: data-parallel over batch; one batch per NeuronCore (8 cores).

Device algorithm per core (optimized for a warm, densely-fed PE):
  Q phase: whole-image pointwise conv into a zero-padded SBUF buffer
    (130x130 per chunk), 9-tap depthwise via diag-matmuls with stride-2
    views -> qd; per-128-col-block PE transposes -> gram accumulate.
  Softmax: row-norm scaling, blockwise softmax over all 24-col blocks,
    block-diagonal mask, Wf = blockdiag(A)^T-contraction with WpT.
  V phase (overlaps softmax): pointwise -> padded buffer -> 9-tap
    depthwise -> fp16 stage -> fused final matmul with Wf -> DMA out.
  All weight transposes / diag-tap matrices / masks precomputed on host.
"""

import numpy as np

C = 192
H = W = 128
HW = H * W
HEADS = 8
CHD = 24
P0, P1 = 128, 64          # channel partition chunks: 0:128 and 128:192
PB = 130                  # padded row width / height (1 + 128 + 1)
PBSZ = PB * PB            # padded image cols per chunk
SUB = 512                 # output subtile cols (4 image rows)
NSUB = HW // SUB          # 32
CHUNK = 2048              # x DMA chunk cols (16 image rows)
NCH = HW // CHUNK         # 8
QSUB = 8                  # qd subtiles (8 rows of 64 each)
TAPS = [(di, dj) for di in range(3) for dj in range(3)]

_BUILT = {}


def _build(iters=1):
    import concourse.mybir as mybir
    import concourse.tile as tile
    from concourse import bacc

    f32 = mybir.dt.float32
    f16 = mybir.dt.float16
    Alu = mybir.AluOpType
    Act = mybir.ActivationFunctionType
    Ax = mybir.AxisListType

    nc = bacc.Bacc(
        "TRN2", target_bir_lowering=False, debug=False,
        enable_asserts=False, num_devices=8,
    )

    # DRAM I/O (per-core shapes)
    xb = nc.dram_tensor("xb", (C, HW), f16, kind="ExternalInput").ap()
    wq = nc.dram_tensor("wq", (P0, 512), f16, kind="ExternalInput").ap()
    wv = nc.dram_tensor("wv", (P0, 512), f16, kind="ExternalInput").ap()
    wp = nc.dram_tensor("wp", (P0, 384), f16, kind="ExternalInput").ap()
    dq = nc.dram_tensor("dq", (P0, 9 * P0), f16, kind="ExternalInput").ap()
    dq1 = nc.dram_tensor("dq1", (P0, 6 * P0), f16, kind="ExternalInput").ap()
    dv = nc.dram_tensor("dv", (P0, 9 * P0), f16, kind="ExternalInput").ap()
    dv1 = nc.dram_tensor("dv1", (P0, 6 * P0), f16, kind="ExternalInput").ap()
    tq = nc.dram_tensor("tq", (C, 1), f32, kind="ExternalInput").ap()
    eye = nc.dram_tensor("eye", (P0, P0), f16, kind="ExternalInput").ap()
    mk0 = nc.dram_tensor("mk0", (P0, C), f32, kind="ExternalInput").ap()
    mk1 = nc.dram_tensor("mk1", (P1, C), f32, kind="ExternalInput").ap()
    out = nc.dram_tensor("out", (C, HW), f32, kind="ExternalOutput").ap()
    import os
    _dbg = os.environ.get("KDBG") == "1"
    if _dbg:
        dqd = nc.dram_tensor("dqd", (P0, 8192), f16, kind="ExternalOutput").ap()
        datt0 = nc.dram_tensor("datt0", (P0, C), f32, kind="ExternalOutput").ap()
        datt1 = nc.dram_tensor("datt1", (P1, C), f32, kind="ExternalOutput").ap()
        dwf = nc.dram_tensor("dwf", (P0, 512), f16, kind="ExternalOutput").ap()
        dgram = nc.dram_tensor("dgram", (P0, C), f32, kind="ExternalOutput").ap()
        dpb = nc.dram_tensor("dpb", (P0, 2 * PBSZ), f16, kind="ExternalOutput").ap()

    import contextlib

    with tile.TileContext(nc) as tc:
      with (tc.For_i(0, iters, 1) if iters > 1 else contextlib.nullcontext()):
        with (
            tc.tile_pool(name="const", bufs=1) as cp,
            tc.tile_pool(name="work", bufs=5) as wkp,
            tc.tile_pool(name="vst", bufs=12) as vsp,
            tc.tile_pool(name="ost", bufs=2) as osp,
            tc.tile_pool(name="psPW", bufs=4, space="PSUM") as psPW,
            tc.tile_pool(name="psT0", bufs=2, space="PSUM") as psT0,
            tc.tile_pool(name="psT1", bufs=2, space="PSUM") as psT1,
        ):
            # ---- constants ----
            # pw weights packed channel-padded to 256: [:,0:256]=WT_pad rows
            # 0:128 (K-chunk0), [:,256:512]=rows 128:256 (K-chunk1, rows 64:128
            # zero).  Uniform [128,128] lhsT slices keep the PE weight loads
            # pipelined (mixed tile shapes serialize LDWEIGHTS).
            wq_sb = cp.tile([P0, 512], f16)
            wv_sb = cp.tile([P0, 512], f16)
            wp_sb = cp.tile([P0, 384], f16)   # WpT rows 0:128 | rows 128:192
            dq_sb = cp.tile([P0, 9 * P0], f16)
            dq1_sb = cp.tile([P0, 6 * P0], f16)  # ch1 paired-tap matrices
            dv_sb = cp.tile([P0, 9 * P0], f16)
            dv1_sb = cp.tile([P0, 6 * P0], f16)
            tq_sb = cp.tile([P0, 2], f32)     # [:,0]=ch0..127, [0:64,1]=ch128..191
            eye_sb = cp.tile([P0, P0], f16)
            mk0_sb = cp.tile([P0, C], f32)    # blockdiag mask rows 0:128
            mk1_sb = cp.tile([P1, C], f32)    # rows 128:192
            pb = cp.tile([P0, 2 * PBSZ], f16)  # padded image: ch0 | ch1
            qd_sb = cp.tile([P0, 8192], f16)  # qd: [:,0:4096] | [0:64,4096:8192]
            g0a = cp.tile([P0, C], f32)       # gram accumulator rows 0:128
            g1a = cp.tile([P1, C], f32)       # rows 128:192
            srow = cp.tile([P0, C], f32)      # s_d broadcast to all partitions
            wf_sb = cp.tile([P0, 512], f16)   # WfT M-padded: K0 @0:256, K1 @256:512
            A0h = cp.tile([P0, C], f16)       # masked softmax attn (fp16)
            A1h = cp.tile([P1, C], f16)
            ssq = cp.tile([P0, 2 * QSUB], f32)  # row sum-of-squares per subtile
            stats = cp.tile([P0, 4 * HEADS], f32)  # softmax stats
            rn = cp.tile([P0, 2], f32)        # 1/||q|| * sqrt(temp)
            scr = cp.tile([P0, SUB], f32)     # scratch for sumsq STT

            pbv0 = pb[:, 0:PBSZ].rearrange("p (r c) -> p r c", c=PB)
            pbv1 = pb[0:P1, PBSZ:2 * PBSZ].rearrange("p (r c) -> p r c", c=PB)
            pbf1 = pb[:, PBSZ:2 * PBSZ].rearrange("p (r c) -> p r c", c=PB)

            # manual 3-slot x ring: ch1 junk partitions zeroed once (on DVE,
            # which is idle at the start) so the channel-padded pw matmuls
            # never stream NaN bit patterns.
            xslots = []
            for _i in range(3):
                xs_t = cp.tile([P0, 2 * CHUNK], f16, name=f"xslot{_i}")
                xslots.append(xs_t)
            for xs in xslots:
                nc.vector.memset(xs[P1:P0, CHUNK:2 * CHUNK], 0.0)

            def dma_xchunk(c):
                xt = xslots[c % 3]
                cs = slice(c * CHUNK, (c + 1) * CHUNK)
                nc.sync.dma_start(out=xt[:, 0:CHUNK], in_=xb[0:P0, cs])
                nc.sync.dma_start(out=xt[0:P1, CHUNK:2 * CHUNK], in_=xb[P0:C, cs])
                return xt

            # HAM warm-up: dependency-free matmuls into dead PSUM slots keep
            # the PE busy through its 3.4us activity window while the first
            # DMAs land, so the real stream starts at full clock.
            for wu in range(16):
                pwu = (psT0 if wu % 2 == 0 else psT1).tile(
                    [P0, 256], f32, tag="t", name="warm")
                nc.tensor.matmul(pwu[:], pb[:, 0:P0], pb[:, 1024:1280],
                                 start=True, stop=True)

            # the first pw matmuls need only wq + x chunk 0, and each
            # dma_start costs ~600ns of Sync issue time: those go first.
            nc.sync.dma_start(out=wq_sb[:], in_=wq[:])
            xq = [dma_xchunk(0)]
            xq.append(dma_xchunk(1))
            nc.sync.dma_start(out=dq_sb[:], in_=dq[:])
            nc.sync.dma_start(out=dq1_sb[:], in_=dq1[:])
            nc.sync.dma_start(out=wv_sb[:], in_=wv[:])
            nc.sync.dma_start(out=wp_sb[:, 0:192], in_=wp[:, 0:192])
            nc.sync.dma_start(out=wp_sb[0:P1, 192:384], in_=wp[0:P1, 192:384])
            nc.sync.dma_start(out=dv_sb[:], in_=dv[:])
            nc.sync.dma_start(out=dv1_sb[:], in_=dv1[:])
            nc.sync.dma_start(out=tq_sb[:, 0:1], in_=tq[0:P0, :])
            nc.sync.dma_start(out=tq_sb[0:P1, 1:2], in_=tq[P0:C, :])
            nc.sync.dma_start(out=eye_sb[:], in_=eye[:])
            nc.sync.dma_start(out=mk0_sb[:], in_=mk0[:])
            nc.sync.dma_start(out=mk1_sb[:], in_=mk1[:])

            # zero the pad rows/cols once; the ch1 region is padded across
            # all 128 partitions (its interior rows 64:128 are rewritten with
            # real zeros by every full-partition pw copy).
            nc.gpsimd.memset(pbv0[:, :, 0:1], 0.0)
            nc.gpsimd.memset(pbv0[:, :, PB - 1:PB], 0.0)
            nc.gpsimd.memset(pbv0[:, 0, :], 0.0)
            nc.gpsimd.memset(pbv0[:, PB - 1, :], 0.0)
            nc.gpsimd.memset(pbf1[:, :, 0:1], 0.0)
            nc.gpsimd.memset(pbf1[:, :, PB - 1:PB], 0.0)
            nc.gpsimd.memset(pbf1[:, 0, :], 0.0)
            nc.gpsimd.memset(pbf1[:, PB - 1, :], 0.0)
            # upper duplicate stores image col m at buffer col m; its cols
            # 128..129 are never written and must stay zero
            nc.gpsimd.memset(pbf1[P1:P0, :, P0:PB - 1], 0.0)

            nc.gpsimd.memset(g0a[:], 0.0)
            nc.gpsimd.memset(g1a[:], 0.0)
            nc.gpsimd.memset(wf_sb[:], 0.0)   # zero rows/cols of the padding

            def pw_subtile(s, xt, s4, w_sb, act_first):
                """Pointwise conv of image rows 4s..4s+3 into padded buffer.
                All 4 matmuls use uniform [128,128] lhsT (channel-padded);
                junk rows 64:128 of the ch1 rhs meet zero weight rows."""
                xr0 = xt[:, s4 * SUB:(s4 + 1) * SUB]
                xr1 = xt[:, CHUNK + s4 * SUB:CHUNK + (s4 + 1) * SUB]
                pp0 = psPW.tile([P0, SUB], f32, tag="pw")
                pp1 = psPW.tile([P0, SUB], f32, tag="pw")
                nc.tensor.matmul(pp0[:], w_sb[:, 0:128], xr0, start=True, stop=False)
                nc.tensor.matmul(pp1[:], w_sb[:, 128:256], xr0, start=True, stop=False)
                nc.tensor.matmul(pp0[:], w_sb[:, 256:384], xr1, start=False, stop=True)
                nc.tensor.matmul(pp1[:], w_sb[:, 384:512], xr1, start=False, stop=True)
                r0 = 4 * s + 1   # buffer row of image row 4s
                d0 = pbv0[:, r0:r0 + 4, 1:129]
                # ch1 lands twice: lower copy at the standard +1 offset and
                # the duplicated upper partitions shifted one column left,
                # which lets two horizontal taps share one matmul.
                d1a = pbf1[0:P1, r0:r0 + 4, 1:129]
                d1b = pbf1[P1:P0, r0:r0 + 4, 0:128]
                v0 = pp0[:].rearrange("p (r c) -> p r c", c=W)
                v1 = pp1[:].rearrange("p (r c) -> p r c", c=W)
                if act_first:
                    nc.scalar.copy(d0, v0)
                    nc.vector.tensor_copy(d1a, v1[0:P1])
                    nc.vector.tensor_copy(d1b, v1[P1:P0])
                else:
                    nc.vector.tensor_copy(d0, v0)
                    nc.scalar.copy(d1a, v1[0:P1])
                    nc.scalar.copy(d1b, v1[P1:P0])

            # ================= Q phase =================
            # pointwise runs one chunk ahead of the taps, and the gram of
            # chunk c runs one iteration late so its transposes never wait
            # on the freshly-written qd copies.
            for s4 in range(4):
                pw_subtile(s4, xq[0], s4, wq_sb, s4 % 2 == 0)

            def emit_gram(k):
                """Gram contribution of qd subtile k's 4 128-col blocks
                (matmuls batched by lhsT shape to keep weight loads
                pipelined)."""
                pg0 = psT0.tile([P0, C], f32, tag="t")
                pg1 = psT1.tile([P1, C], f32, tag="t")
                qdTs = []
                for b4 in range(4):
                    kcol = k * SUB + b4 * P0
                    pt0 = psPW.tile([P0, P0], f16, tag="pw")
                    pt1 = psPW.tile([P0, P1], f16, tag="pw")
                    nc.tensor.transpose(pt0[:], qd_sb[:, kcol:kcol + P0], eye_sb[:])
                    nc.tensor.transpose(pt1[:], qd_sb[0:P1, 4096 + kcol:4096 + kcol + P0],
                                        eye_sb[0:P1, 0:P1])
                    qdT = wkp.tile([P0, C], f16, tag="qdT")
                    nc.vector.tensor_copy(qdT[:, 0:P0], pt0[:])
                    nc.vector.tensor_copy(qdT[:, P0:C], pt1[:])
                    qdTs.append(qdT)
                # the masked softmax only reads the diagonal head blocks:
                # rows 0:128 need cols 0:144 (heads 0-5), rows 128:192 need
                # cols 120:192 (heads 5-7); the rest of g0a/g1a stays zero.
                for b4 in range(4):
                    nc.tensor.matmul(pg0[:, 0:144], qdTs[b4][:, 0:P0],
                                     qdTs[b4][:, 0:144],
                                     start=(b4 == 0), stop=(b4 == 3))
                for b4 in range(4):
                    nc.tensor.matmul(pg1[:, 0:72], qdTs[b4][:, P0:C],
                                     qdTs[b4][:, 120:192],
                                     start=(b4 == 0), stop=(b4 == 3))
                nc.vector.tensor_tensor(g0a[:, 0:144], g0a[:, 0:144],
                                        pg0[:, 0:144], Alu.add)
                nc.vector.tensor_tensor(g1a[:, 120:192], g1a[:, 120:192],
                                        pg1[:, 0:72], Alu.add)

            for c in range(NCH):
                if c + 1 < NCH:
                    for s4 in range(4):
                        pw_subtile(4 * (c + 1) + s4, xq[c + 1], s4, wq_sb,
                                   s4 % 2 == 0)
                    if c + 2 < NCH:
                        xq.append(dma_xchunk(c + 2))
                # taps for qd subtile k=c (qd rows 8c..8c+7); ch1 uses the
                # zero-padded [128,128] diagonals on the full-partition view
                k = c
                pq0 = psT0.tile([P0, SUB], f32, tag="t")
                pq1 = psT1.tile([P0, SUB], f32, tag="t")
                o0 = pq0[:].rearrange("p (r c) -> p r c", c=64)
                o1 = pq1[:].rearrange("p (r c) -> p r c", c=64)
                rb = 16 * k  # buffer row of qd row 8k input base (2i, i=8k)
                for t, (di, dj) in enumerate(TAPS):
                    rhs0 = pbv0[:, rb + di:rb + di + 16:2, dj:dj + 128:2]
                    nc.tensor.matmul(o0, dq_sb[:, t * P0:(t + 1) * P0], rhs0,
                                     start=(t == 0), stop=(t == 8))
                for g in range(6):
                    di, dj = (g, 0) if g < 3 else (g - 3, 2)
                    rhs1 = pbf1[:, rb + di:rb + di + 16:2, dj:dj + 128:2]
                    nc.tensor.matmul(o1, dq1_sb[:, g * P0:(g + 1) * P0], rhs1,
                                     start=(g == 0), stop=(g == 5))
                nc.scalar.activation(scr[:], pq0[:], Act.Square,
                                     accum_out=ssq[:, k:k + 1])
                nc.scalar.activation(scr[0:P1, :], pq1[0:P1, :], Act.Square,
                                     accum_out=ssq[0:P1, QSUB + k:QSUB + k + 1])
                nc.vector.tensor_copy(qd_sb[:, k * SUB:(k + 1) * SUB], pq0[:])
                nc.vector.tensor_copy(qd_sb[0:P1, 4096 + k * SUB:4096 + (k + 1) * SUB],
                                      pq1[0:P1, :])
                if c >= 1:
                    emit_gram(c - 1)
            emit_gram(NCH - 1)

            if _dbg:
                nc.sync.dma_start(out=dqd, in_=qd_sb[:])
                nc.sync.dma_start(out=dgram, in_=g0a[:])
                nc.sync.dma_start(out=dpb, in_=pb[:])
            # ================= V phase helpers =================
            # taps for subtile j need pad rows 4j..4j+5; row 4j+5 is written
            # by pw subtile j+1, so taps lag the pointwise by one subtile.
            def emit_taps_v(j):
                pv0 = psT0.tile([P0, SUB], f32, tag="t")
                pv1 = psT1.tile([P0, SUB], f32, tag="t")
                o0 = pv0[:].rearrange("p (r c) -> p r c", c=W)
                o1 = pv1[:].rearrange("p (r c) -> p r c", c=W)
                rb = 4 * j
                for t, (di, dj) in enumerate(TAPS):
                    rhs0 = pbv0[:, rb + di:rb + di + 4, dj:dj + 128]
                    nc.tensor.matmul(o0, dv_sb[:, t * P0:(t + 1) * P0], rhs0,
                                     start=(t == 0), stop=(t == 8))
                for g in range(6):
                    di, dj = (g, 0) if g < 3 else (g - 3, 2)
                    rhs1 = pbf1[:, rb + di:rb + di + 4, dj:dj + 128]
                    nc.tensor.matmul(o1, dv1_sb[:, g * P0:(g + 1) * P0], rhs1,
                                     start=(g == 0), stop=(g == 5))
                vst0 = vsp.tile([P0, SUB], f16, tag="v0")
                vst1 = vsp.tile([P0, SUB], f16, tag="v1")
                nc.vector.tensor_copy(vst0[:], pv0[:])
                nc.vector.tensor_copy(vst1[:], pv1[:])
                return vst0, vst1

            och = {}

            def emit_final(j, vst0, vst1):
                """Fused final matmul: out = WfT-contraction @ v_dw.  All 4
                lhsT slices are uniform [128,128]; vst1 junk rows 64:128 meet
                the zero rows of the padded Wf."""
                c, s4 = j // 4, j % 4
                if s4 == 0:
                    och[c] = (osp.tile([P0, CHUNK], f32, tag="o0", name="och0"),
                              osp.tile([P1, CHUNK], f32, tag="o1", name="och1"))
                och0, och1 = och[c]
                po0 = psT0.tile([P0, SUB], f32, tag="t")
                po1 = psT1.tile([P0, SUB], f32, tag="t")
                nc.tensor.matmul(po0[:], wf_sb[:, 0:128], vst0[:],
                                 start=True, stop=False)
                nc.tensor.matmul(po1[:], wf_sb[:, 128:256], vst0[:],
                                 start=True, stop=False)
                nc.tensor.matmul(po0[:], wf_sb[:, 256:384], vst1[:],
                                 start=False, stop=True)
                nc.tensor.matmul(po1[:], wf_sb[:, 384:512], vst1[:],
                                 start=False, stop=True)
                nc.scalar.copy(och0[:, s4 * SUB:(s4 + 1) * SUB], po0[:])
                nc.vector.tensor_copy(och1[:, s4 * SUB:(s4 + 1) * SUB],
                                      po1[0:P1, :])
                if c == NCH - 1:
                    # last chunk: flush per subtile so the tail DMA is short
                    scs = slice(c * CHUNK + s4 * SUB, c * CHUNK + (s4 + 1) * SUB)
                    ssl = slice(s4 * SUB, (s4 + 1) * SUB)
                    nc.sync.dma_start(out=out[0:P0, scs], in_=och0[:, ssl])
                    nc.sync.dma_start(out=out[P0:C, scs], in_=och1[:, ssl])
                elif s4 == 3:
                    ocs = slice(c * CHUNK, (c + 1) * CHUNK)
                    nc.sync.dma_start(out=out[0:P0, ocs], in_=och0[:])
                    nc.sync.dma_start(out=out[P0:C, ocs], in_=och1[:])

            # Pre-emit the first two V chunks' pointwise and six tap groups
            # so the in-order PE queue has work while the softmax chain runs.
            xv = [dma_xchunk(0)]
            for s4 in range(4):
                pw_subtile(s4, xv[0], s4, wv_sb, s4 % 2 == 0)
            xv.append(dma_xchunk(1))
            xv.append(dma_xchunk(2))
            for s4 in range(4):
                pw_subtile(4 + s4, xv[1], s4, wv_sb, s4 % 2 == 0)
            xv.append(dma_xchunk(3))
            pend = [(j,) + emit_taps_v(j) for j in range(6)]

            # ---- row scales: rn = sqrt(temp) / ||qd_row|| ----
            # ACT Sqrt is low-precision; one Newton step on y=sqrt(ss).
            nc.vector.tensor_reduce(ssq[:, 0:1], ssq[:, 0:QSUB], Ax.X, Alu.add)
            nc.vector.tensor_reduce(ssq[0:P1, QSUB:QSUB + 1],
                                    ssq[0:P1, QSUB:2 * QSUB], Ax.X, Alu.add)
            for ss_ap, rn_ap, tq_ap in (
                (ssq[:, 0:1], rn[:, 0:1], tq_sb[:, 0:1]),
                (ssq[0:P1, QSUB:QSUB + 1], rn[0:P1, 1:2], tq_sb[0:P1, 1:2]),
            ):
                y = scr[0:ss_ap.shape[0], 0:1]
                yr = scr[0:ss_ap.shape[0], 1:2]
                nc.scalar.activation(y, ss_ap, Act.Sqrt)
                nc.vector.reciprocal(yr, y)                      # 1/y
                nc.vector.tensor_tensor(yr, yr, ss_ap, Alu.mult)  # ss/y
                nc.vector.tensor_tensor(y, y, yr, Alu.add)
                nc.vector.tensor_scalar_mul(y, y, 0.5)            # refined sqrt
                nc.vector.reciprocal(rn_ap, y)
                nc.vector.tensor_tensor(rn_ap, rn_ap, tq_ap, Alu.mult)

            # attn = diag(s) G diag(s): row scale by s_c, then elementwise
            # multiply by s_d replicated across partitions.
            nc.sync.dma_start(out=srow[0:1, 0:P0], in_=rn[:, 0:1])
            nc.sync.dma_start(out=srow[0:1, P0:C], in_=rn[0:P1, 1:2])
            nc.gpsimd.partition_broadcast(srow[:], srow[0:1, :])
            nc.vector.tensor_scalar_mul(g0a[:], g0a[:], rn[:, 0:1])
            nc.vector.tensor_scalar_mul(g1a[:], g1a[:], rn[0:P1, 1:2])
            nc.vector.tensor_tensor(g0a[:], g0a[:], srow[:], Alu.mult)
            nc.vector.tensor_tensor(g1a[:], g1a[:], srow[0:P1, :], Alu.mult)

            # ---- blockwise softmax over every 24-col block, then keep the
            # diagonal block per row via a precomputed mask ----
            gv0 = g0a[:].rearrange("p (h c) -> p h c", c=CHD)
            gv1 = g1a[:].rearrange("p (h c) -> p h c", c=CHD)
            # logits are cosine-similarities scaled by temperature (=1):
            # bounded, so exp is safe without the max-subtraction pass
            nc.scalar.activation(g0a[:], g0a[:], Act.Exp)
            nc.scalar.activation(g1a[:], g1a[:], Act.Exp)
            sm0 = stats[:, 2 * HEADS:3 * HEADS]
            sm1 = stats[0:P1, 3 * HEADS:4 * HEADS]
            nc.vector.tensor_reduce(sm0, gv0, Ax.X, Alu.add)
            nc.vector.tensor_reduce(sm1, gv1, Ax.X, Alu.add)
            nc.vector.reciprocal(sm0, sm0)
            nc.vector.reciprocal(sm1, sm1)
            nc.vector.tensor_tensor(gv0, gv0,
                                    sm0.unsqueeze(2).broadcast_to((P0, HEADS, CHD)),
                                    Alu.mult)
            nc.vector.tensor_tensor(gv1, gv1,
                                    sm1.unsqueeze(2).broadcast_to((P1, HEADS, CHD)),
                                    Alu.mult)
            nc.vector.tensor_tensor(g0a[:], g0a[:], mk0_sb[:], Alu.mult)
            nc.vector.tensor_tensor(g1a[:], g1a[:], mk1_sb[:], Alu.mult)
            nc.vector.tensor_copy(A0h[:], g0a[:])
            nc.vector.tensor_copy(A1h[:], g1a[:])
            if _dbg:
                nc.sync.dma_start(out=datt0, in_=g0a[:])
                nc.sync.dma_start(out=datt1, in_=g1a[:])

            # ---- WfT = blockdiag(A)-contraction with WpT (fp16) ----
            pwf0 = psT0.tile([P0, C], f32, tag="t")
            pwf1 = psT1.tile([P1, C], f32, tag="t")
            nc.tensor.matmul(pwf0[:], A0h[:, 0:P0], wp_sb[:, 0:192],
                             start=True, stop=False)
            nc.tensor.matmul(pwf0[:], A1h[:, 0:P0], wp_sb[0:P1, 192:384],
                             start=False, stop=True)
            nc.tensor.matmul(pwf1[:], A0h[:, P0:C], wp_sb[:, 0:192],
                             start=True, stop=False)
            nc.tensor.matmul(pwf1[:], A1h[:, P0:C], wp_sb[0:P1, 192:384],
                             start=False, stop=True)
            nc.scalar.copy(wf_sb[:, 0:192], pwf0[:])
            nc.scalar.copy(wf_sb[0:P1, 256:448], pwf1[:])
            if _dbg:
                nc.sync.dma_start(out=dwf, in_=wf_sb[:])

            # ================= V phase (remainder) =================
            for jv, v0p, v1p in pend:
                emit_final(jv, v0p, v1p)
            for s in range(7, NSUB + 1):
                if 8 <= s < NSUB:
                    pw_subtile(s, xv[s // 4], s % 4, wv_sb, s % 2 == 0)
                    if s % 4 == 3 and s // 4 + 2 < NCH:
                        xv.append(dma_xchunk(s // 4 + 2))
                j = s - 1
                v0p, v1p = emit_taps_v(j)
                emit_final(j, v0p, v1p)

    nc.compile()
    return nc


def _host_inputs(x, w_qkv, w_dw, w_proj, temperature):
    """Per-core input maps (host-side precompute of all weight transforms)."""
    f = np.float32
    W_q = w_qkv[0:C].astype(f)           # (192,192)
    W_v = w_qkv[2 * C:3 * C].astype(f)
    wq_d = w_dw[0:C, 0].reshape(C, 9).astype(f)        # (192,9) taps (di,dj)
    wv_d = w_dw[2 * C:3 * C, 0].reshape(C, 9).astype(f)

    def pack_T(Wm):
        """W^T channel-padded to 256 and packed as (128, 512):
        [:,0:256]=rows 0:128 of WT_pad (K-chunk0), [:,256:512]=rows 128:256
        (K-chunk1; rows 64:128 zero).  Output channels 192:256 duplicate
        128:192 so the ch1 image lands twice in the padded buffer, enabling
        paired depthwise taps."""
        WTp = np.zeros((256, 256), f)
        WTp[0:C, 0:C] = Wm.T.astype(f)
        WTp[:, 192:256] = WTp[:, 128:192]
        return np.concatenate([WTp[0:P0], WTp[P0:256]], axis=1)

    def pack_diag(wd, lo, n):
        """9 diagonal tap matrices, zero-padded to (128, 9*128)."""
        out = np.zeros((P0, 9 * P0), f)
        for t in range(9):
            np.fill_diagonal(out[0:n, t * P0:t * P0 + n], wd[lo:lo + n, t])
        return out

    def pack_diag1(wd):
        """ch1 tap matrices for the duplicated layout, (128, 6*128):
        g<3: paired taps (di,dj=0)+(di,dj=1) -- rows 0:64 scale the lower
        copy, rows 64:128 the col-shifted upper copy, both into out 0:64;
        g>=3: single tap (di,dj=2) on the lower copy only."""
        out = np.zeros((P0, 6 * P0), f)
        w = wd[P0:C].reshape(P1, 3, 3)
        for di in range(3):
            np.fill_diagonal(out[0:P1, di * P0:di * P0 + P1], w[:, di, 0])
            np.fill_diagonal(out[P1:P0, di * P0:di * P0 + P1], w[:, di, 1])
            g = 3 + di
            np.fill_diagonal(out[0:P1, g * P0:g * P0 + P1], w[:, di, 2])
        return out

    wp_pack = np.zeros((P0, 384), f)
    WpT = w_proj.T.astype(f)
    wp_pack[:, 0:192] = WpT[0:P0]
    wp_pack[0:P1, 192:384] = WpT[P0:C]

    tq = np.sqrt(np.repeat(temperature.reshape(HEADS).astype(f), CHD)).reshape(C, 1)
    eye = np.eye(P0, dtype=np.float16)

    heads = np.arange(C) // CHD
    mask_full = (heads[:, None] == heads[None, :]).astype(f)   # (192,192)

    shared = {
        "wq": pack_T(W_q), "wv": pack_T(W_v), "wp": wp_pack,
        "dq": pack_diag(wq_d, 0, P0), "dq1": pack_diag1(wq_d),
        "dv": pack_diag(wv_d, 0, P0), "dv1": pack_diag1(wv_d),
        "tq": tq, "eye": eye.astype(np.float16),
        "mk0": np.ascontiguousarray(mask_full[0:P0]),
        "mk1": np.ascontiguousarray(mask_full[P0:C]),
    }
    h = np.float16
    for k in ("wq", "wv", "wp", "dq", "dq1", "dv", "dv1"):
        shared[k] = shared[k].astype(h)
    maps = []
    for b in range(8):
        m = dict(shared)
        m["xb"] = np.ascontiguousarray(x[b].reshape(C, HW).astype(h))
        maps.append(m)
    return maps


def kernel(x, w_qkv, w_dw, w_proj, temperature, _trace=False, _iters=1):
    from concourse.bass_utils import run_bass_kernel_spmd
    if _iters not in _BUILT:
        _BUILT[_iters] = _build(_iters)
    nc = _BUILT[_iters]
    in_maps = _host_inputs(
        np.asarray(x), np.asarray(w_qkv), np.asarray(w_dw),
        np.asarray(w_proj), np.asarray(temperature))
    res = run_bass_kernel_spmd(nc, in_maps, list(range(8)), trace=_trace)
    outs = [res.results[i]["out"].reshape(C, H, W) for i in range(8)]
    y = np.stack(outs, axis=0).astype(np.float32)
    kernel.last_result = res
    return y


# revision 70
# speedup vs baseline: 1.0411x; 1.0008x over previous
"""Trainium2 Bass kernel for nn_Attention_45749991637079.

Reference computation (per batch b, C=192 channels, 128x128 image, 8 heads):
    qkv  = w_qkv @ x                       (1x1 conv; k-branch unused)
    q,v  = depthwise 3x3 (SAME) of the q/v channel blocks
    qd   = q[:, ::2, ::2]                  (64x64 downsample)
    attn = softmax(l2norm-rows(qd_h) gram * temp)   per head (24x24)
    out  = w_proj @ (attn @ v)             == (w_proj @ blockdiag(attn)) @ v

Sharding: data-parallel over batch; one batch per NeuronCore (8 cores).

Device algorithm per core (optimized for a warm, densely-fed PE):
  Q phase: whole-image pointwise conv into a zero-padded SBUF buffer
    (130x130 per chunk), 9-tap depthwise via diag-matmuls with stride-2
    views -> qd; per-128-col-block PE transposes -> gram accumulate.
  Softmax: row-norm scaling, blockwise softmax over all 24-col blocks,
    block-diagonal mask, Wf = blockdiag(A)^T-contraction with WpT.
  V phase (overlaps softmax): pointwise -> padded buffer -> 9-tap
    depthwise -> fp16 stage -> fused final matmul with Wf -> DMA out.
  All weight transposes / diag-tap matrices / masks precomputed on host.
"""

import numpy as np

C = 192
H = W = 128
HW = H * W
HEADS = 8
CHD = 24
P0, P1 = 128, 64          # channel partition chunks: 0:128 and 128:192
PB = 130                  # padded row width / height (1 + 128 + 1)
PBSZ = PB * PB            # padded image cols per chunk
SUB = 512                 # output subtile cols (4 image rows)
NSUB = HW // SUB          # 32
CHUNK = 2048              # x DMA chunk cols (16 image rows)
NCH = HW // CHUNK         # 8
QSUB = 8                  # qd subtiles (8 rows of 64 each)
TAPS = [(di, dj) for di in range(3) for dj in range(3)]

_BUILT = {}


def _build(iters=1):
    import concourse.mybir as mybir
    import concourse.tile as tile
    from concourse import bacc

    f32 = mybir.dt.float32
    f16 = mybir.dt.float16
    Alu = mybir.AluOpType
    Act = mybir.ActivationFunctionType
    Ax = mybir.AxisListType

    nc = bacc.Bacc(
        "TRN2", target_bir_lowering=False, debug=False,
        enable_asserts=False, num_devices=8,
    )

    # DRAM I/O (per-core shapes)
    xb = nc.dram_tensor("xb", (C, HW), f16, kind="ExternalInput").ap()
    wq = nc.dram_tensor("wq", (P0, 512), f16, kind="ExternalInput").ap()
    wv = nc.dram_tensor("wv", (P0, 512), f16, kind="ExternalInput").ap()
    wp = nc.dram_tensor("wp", (P0, 384), f16, kind="ExternalInput").ap()
    dq = nc.dram_tensor("dq", (P0, 9 * P0), f16, kind="ExternalInput").ap()
    dq1 = nc.dram_tensor("dq1", (P0, 6 * P0), f16, kind="ExternalInput").ap()
    dv = nc.dram_tensor("dv", (P0, 9 * P0), f16, kind="ExternalInput").ap()
    dv1 = nc.dram_tensor("dv1", (P0, 6 * P0), f16, kind="ExternalInput").ap()
    tq = nc.dram_tensor("tq", (C, 1), f32, kind="ExternalInput").ap()
    eye = nc.dram_tensor("eye", (P0, P0), f16, kind="ExternalInput").ap()
    mk0 = nc.dram_tensor("mk0", (P0, C), f32, kind="ExternalInput").ap()
    mk1 = nc.dram_tensor("mk1", (P1, C), f32, kind="ExternalInput").ap()
    out = nc.dram_tensor("out", (C, HW), f32, kind="ExternalOutput").ap()
    import os
    _dbg = os.environ.get("KDBG") == "1"
    if _dbg:
        dqd = nc.dram_tensor("dqd", (P0, 8192), f16, kind="ExternalOutput").ap()
        datt0 = nc.dram_tensor("datt0", (P0, C), f32, kind="ExternalOutput").ap()
        datt1 = nc.dram_tensor("datt1", (P1, C), f32, kind="ExternalOutput").ap()
        dwf = nc.dram_tensor("dwf", (P0, 512), f16, kind="ExternalOutput").ap()
        dgram = nc.dram_tensor("dgram", (P0, C), f32, kind="ExternalOutput").ap()
        dpb = nc.dram_tensor("dpb", (P0, 2 * PBSZ), f16, kind="ExternalOutput").ap()

    import contextlib

    with tile.TileContext(nc) as tc:
      with (tc.For_i(0, iters, 1) if iters > 1 else contextlib.nullcontext()):
        with (
            tc.tile_pool(name="const", bufs=1) as cp,
            tc.tile_pool(name="work", bufs=5) as wkp,
            tc.tile_pool(name="vst", bufs=12) as vsp,
            tc.tile_pool(name="ost", bufs=2) as osp,
            tc.tile_pool(name="psPW", bufs=4, space="PSUM") as psPW,
            tc.tile_pool(name="psT0", bufs=2, space="PSUM") as psT0,
            tc.tile_pool(name="psT1", bufs=2, space="PSUM") as psT1,
        ):
            # ---- constants ----
            # pw weights packed channel-padded to 256: [:,0:256]=WT_pad rows
            # 0:128 (K-chunk0), [:,256:512]=rows 128:256 (K-chunk1, rows 64:128
            # zero).  Uniform [128,128] lhsT slices keep the PE weight loads
            # pipelined (mixed tile shapes serialize LDWEIGHTS).
            wq_sb = cp.tile([P0, 512], f16)
            wv_sb = cp.tile([P0, 512], f16)
            wp_sb = cp.tile([P0, 384], f16)   # WpT rows 0:128 | rows 128:192
            dq_sb = cp.tile([P0, 9 * P0], f16)
            dq1_sb = cp.tile([P0, 6 * P0], f16)  # ch1 paired-tap matrices
            dv_sb = cp.tile([P0, 9 * P0], f16)
            dv1_sb = cp.tile([P0, 6 * P0], f16)
            tq_sb = cp.tile([P0, 2], f32)     # [:,0]=ch0..127, [0:64,1]=ch128..191
            eye_sb = cp.tile([P0, P0], f16)
            mk0_sb = cp.tile([P0, C], f32)    # blockdiag mask rows 0:128
            mk1_sb = cp.tile([P1, C], f32)    # rows 128:192
            pb = cp.tile([P0, 2 * PBSZ], f16)  # padded image: ch0 | ch1
            qd_sb = cp.tile([P0, 8192], f16)  # qd: [:,0:4096] | [0:64,4096:8192]
            g0a = cp.tile([P0, C], f32)       # gram accumulator rows 0:128
            g1a = cp.tile([P1, C], f32)       # rows 128:192
            srow = cp.tile([P0, C], f32)      # s_d broadcast to all partitions
            wf_sb = cp.tile([P0, 512], f16)   # WfT M-padded: K0 @0:256, K1 @256:512
            A0h = cp.tile([P0, C], f16)       # masked softmax attn (fp16)
            A1h = cp.tile([P1, C], f16)
            ssq = cp.tile([P0, 2 * QSUB], f32)  # row sum-of-squares per subtile
            stats = cp.tile([P0, 4 * HEADS], f32)  # softmax stats
            rn = cp.tile([P0, 2], f32)        # 1/||q|| * sqrt(temp)
            scr = cp.tile([P0, SUB], f32)     # scratch for sumsq STT

            pbv0 = pb[:, 0:PBSZ].rearrange("p (r c) -> p r c", c=PB)
            pbv1 = pb[0:P1, PBSZ:2 * PBSZ].rearrange("p (r c) -> p r c", c=PB)
            pbf1 = pb[:, PBSZ:2 * PBSZ].rearrange("p (r c) -> p r c", c=PB)

            # manual 3-slot x ring: ch1 junk partitions zeroed once (on DVE,
            # which is idle at the start) so the channel-padded pw matmuls
            # never stream NaN bit patterns.
            xslots = []
            for _i in range(3):
                xs_t = cp.tile([P0, 2 * CHUNK], f16, name=f"xslot{_i}")
                xslots.append(xs_t)
            for xs in xslots:
                nc.vector.memset(xs[P1:P0, CHUNK:2 * CHUNK], 0.0)

            def dma_xchunk(c):
                xt = xslots[c % 3]
                cs = slice(c * CHUNK, (c + 1) * CHUNK)
                nc.sync.dma_start(out=xt[:, 0:CHUNK], in_=xb[0:P0, cs])
                nc.sync.dma_start(out=xt[0:P1, CHUNK:2 * CHUNK], in_=xb[P0:C, cs])
                return xt

            # HAM warm-up: dependency-free matmuls into dead PSUM slots keep
            # the PE busy through its 3.4us activity window while the first
            # DMAs land, so the real stream starts at full clock.
            for wu in range(10):
                pwu = (psT0 if wu % 2 == 0 else psT1).tile(
                    [P0, 256], f32, tag="t", name="warm")
                nc.tensor.matmul(pwu[:], pb[:, 0:P0], pb[:, 1024:1280],
                                 start=True, stop=True)

            # the first pw matmuls need only x chunk 0 (the 512KB long pole)
            # + wq, and each dma_start costs ~600ns of Sync issue time:
            # those go first.
            xq = [dma_xchunk(0)]
            nc.sync.dma_start(out=wq_sb[:], in_=wq[:])
            xq.append(dma_xchunk(1))
            nc.sync.dma_start(out=dq_sb[:], in_=dq[:])
            nc.sync.dma_start(out=dq1_sb[:], in_=dq1[:])
            nc.sync.dma_start(out=wv_sb[:], in_=wv[:])
            nc.sync.dma_start(out=wp_sb[:, 0:192], in_=wp[:, 0:192])
            nc.sync.dma_start(out=wp_sb[0:P1, 192:384], in_=wp[0:P1, 192:384])
            nc.sync.dma_start(out=dv_sb[:], in_=dv[:])
            nc.sync.dma_start(out=dv1_sb[:], in_=dv1[:])
            nc.sync.dma_start(out=tq_sb[:, 0:1], in_=tq[0:P0, :])
            nc.sync.dma_start(out=tq_sb[0:P1, 1:2], in_=tq[P0:C, :])
            nc.sync.dma_start(out=eye_sb[:], in_=eye[:])
            nc.sync.dma_start(out=mk0_sb[:], in_=mk0[:])
            nc.sync.dma_start(out=mk1_sb[:], in_=mk1[:])

            # zero the pad rows/cols once; the ch1 region is padded across
            # all 128 partitions (its interior rows 64:128 are rewritten with
            # real zeros by every full-partition pw copy).
            nc.gpsimd.memset(pbv0[:, :, 0:1], 0.0)
            nc.gpsimd.memset(pbv0[:, :, PB - 1:PB], 0.0)
            nc.gpsimd.memset(pbv0[:, 0, :], 0.0)
            nc.gpsimd.memset(pbv0[:, PB - 1, :], 0.0)
            nc.gpsimd.memset(pbf1[:, :, 0:1], 0.0)
            nc.gpsimd.memset(pbf1[:, :, PB - 1:PB], 0.0)
            nc.gpsimd.memset(pbf1[:, 0, :], 0.0)
            nc.gpsimd.memset(pbf1[:, PB - 1, :], 0.0)
            # upper duplicate stores image col m at buffer col m; its cols
            # 128..129 are never written and must stay zero
            nc.gpsimd.memset(pbf1[P1:P0, :, P0:PB - 1], 0.0)

            nc.gpsimd.memset(g0a[:], 0.0)
            nc.gpsimd.memset(g1a[:], 0.0)
            nc.gpsimd.memset(wf_sb[:], 0.0)   # zero rows/cols of the padding

            def pw_subtile(s, xt, s4, w_sb, act_first):
                """Pointwise conv of image rows 4s..4s+3 into padded buffer.
                All 4 matmuls use uniform [128,128] lhsT (channel-padded);
                junk rows 64:128 of the ch1 rhs meet zero weight rows."""
                xr0 = xt[:, s4 * SUB:(s4 + 1) * SUB]
                xr1 = xt[:, CHUNK + s4 * SUB:CHUNK + (s4 + 1) * SUB]
                pp0 = psPW.tile([P0, SUB], f32, tag="pw")
                pp1 = psPW.tile([P0, SUB], f32, tag="pw")
                nc.tensor.matmul(pp0[:], w_sb[:, 0:128], xr0, start=True, stop=False)
                nc.tensor.matmul(pp1[:], w_sb[:, 128:256], xr0, start=True, stop=False)
                nc.tensor.matmul(pp0[:], w_sb[:, 256:384], xr1, start=False, stop=True)
                nc.tensor.matmul(pp1[:], w_sb[:, 384:512], xr1, start=False, stop=True)
                r0 = 4 * s + 1   # buffer row of image row 4s
                d0 = pbv0[:, r0:r0 + 4, 1:129]
                # ch1 lands twice: lower copy at the standard +1 offset and
                # the duplicated upper partitions shifted one column left,
                # which lets two horizontal taps share one matmul.
                d1a = pbf1[0:P1, r0:r0 + 4, 1:129]
                d1b = pbf1[P1:P0, r0:r0 + 4, 0:128]
                v0 = pp0[:].rearrange("p (r c) -> p r c", c=W)
                v1 = pp1[:].rearrange("p (r c) -> p r c", c=W)
                if act_first:
                    nc.scalar.copy(d0, v0)
                    nc.vector.tensor_copy(d1a, v1[0:P1])
                    nc.vector.tensor_copy(d1b, v1[P1:P0])
                else:
                    nc.vector.tensor_copy(d0, v0)
                    nc.scalar.copy(d1a, v1[0:P1])
                    nc.scalar.copy(d1b, v1[P1:P0])

            # ================= Q phase =================
            # pointwise runs one chunk ahead of the taps, and the gram of
            # chunk c runs one iteration late so its transposes never wait
            # on the freshly-written qd copies.
            for s4 in range(4):
                pw_subtile(s4, xq[0], s4, wq_sb, s4 % 2 == 0)

            def emit_gram(k):
                """Gram contribution of qd subtile k's 4 128-col blocks
                (matmuls batched by lhsT shape to keep weight loads
                pipelined)."""
                pg0 = psT0.tile([P0, C], f32, tag="t")
                pg1 = psT1.tile([P1, C], f32, tag="t")
                qdTs = []
                for b4 in range(4):
                    kcol = k * SUB + b4 * P0
                    pt0 = psPW.tile([P0, P0], f16, tag="pw")
                    pt1 = psPW.tile([P0, P1], f16, tag="pw")
                    nc.tensor.transpose(pt0[:], qd_sb[:, kcol:kcol + P0], eye_sb[:])
                    nc.tensor.transpose(pt1[:], qd_sb[0:P1, 4096 + kcol:4096 + kcol + P0],
                                        eye_sb[0:P1, 0:P1])
                    qdT = wkp.tile([P0, C], f16, tag="qdT")
                    nc.vector.tensor_copy(qdT[:, 0:P0], pt0[:])
                    nc.vector.tensor_copy(qdT[:, P0:C], pt1[:])
                    qdTs.append(qdT)
                # the masked softmax only reads the diagonal head blocks:
                # rows 0:128 need cols 0:144 (heads 0-5), rows 128:192 need
                # cols 120:192 (heads 5-7); the rest of g0a/g1a stays zero.
                for b4 in range(4):
                    nc.tensor.matmul(pg0[:, 0:144], qdTs[b4][:, 0:P0],
                                     qdTs[b4][:, 0:144],
                                     start=(b4 == 0), stop=(b4 == 3))
                for b4 in range(4):
                    nc.tensor.matmul(pg1[:, 0:72], qdTs[b4][:, P0:C],
                                     qdTs[b4][:, 120:192],
                                     start=(b4 == 0), stop=(b4 == 3))
                nc.vector.tensor_tensor(g0a[:, 0:144], g0a[:, 0:144],
                                        pg0[:, 0:144], Alu.add)
                nc.vector.tensor_tensor(g1a[:, 120:192], g1a[:, 120:192],
                                        pg1[:, 0:72], Alu.add)

            for c in range(NCH):
                if c + 1 < NCH:
                    for s4 in range(4):
                        pw_subtile(4 * (c + 1) + s4, xq[c + 1], s4, wq_sb,
                                   s4 % 2 == 0)
                    if c + 2 < NCH:
                        xq.append(dma_xchunk(c + 2))
                # taps for qd subtile k=c (qd rows 8c..8c+7); ch1 uses the
                # zero-padded [128,128] diagonals on the full-partition view
                k = c
                pq0 = psT0.tile([P0, SUB], f32, tag="t")
                pq1 = psT1.tile([P0, SUB], f32, tag="t")
                o0 = pq0[:].rearrange("p (r c) -> p r c", c=64)
                o1 = pq1[:].rearrange("p (r c) -> p r c", c=64)
                rb = 16 * k  # buffer row of qd row 8k input base (2i, i=8k)
                for t, (di, dj) in enumerate(TAPS):
                    rhs0 = pbv0[:, rb + di:rb + di + 16:2, dj:dj + 128:2]
                    nc.tensor.matmul(o0, dq_sb[:, t * P0:(t + 1) * P0], rhs0,
                                     start=(t == 0), stop=(t == 8))
                for g in range(6):
                    di, dj = (g, 0) if g < 3 else (g - 3, 2)
                    rhs1 = pbf1[:, rb + di:rb + di + 16:2, dj:dj + 128:2]
                    nc.tensor.matmul(o1, dq1_sb[:, g * P0:(g + 1) * P0], rhs1,
                                     start=(g == 0), stop=(g == 5))
                nc.scalar.activation(scr[:], pq0[:], Act.Square,
                                     accum_out=ssq[:, k:k + 1])
                nc.scalar.activation(scr[0:P1, :], pq1[0:P1, :], Act.Square,
                                     accum_out=ssq[0:P1, QSUB + k:QSUB + k + 1])
                nc.vector.tensor_copy(qd_sb[:, k * SUB:(k + 1) * SUB], pq0[:])
                nc.vector.tensor_copy(qd_sb[0:P1, 4096 + k * SUB:4096 + (k + 1) * SUB],
                                      pq1[0:P1, :])
                if c >= 1:
                    emit_gram(c - 1)
            emit_gram(NCH - 1)

            if _dbg:
                nc.sync.dma_start(out=dqd, in_=qd_sb[:])
                nc.sync.dma_start(out=dgram, in_=g0a[:])
                nc.sync.dma_start(out=dpb, in_=pb[:])
            # ================= V phase helpers =================
            # taps for subtile j need pad rows 4j..4j+5; row 4j+5 is written
            # by pw subtile j+1, so taps lag the pointwise by one subtile.
            def emit_taps_v(j):
                pv0 = psT0.tile([P0, SUB], f32, tag="t")
                pv1 = psT1.tile([P0, SUB], f32, tag="t")
                o0 = pv0[:].rearrange("p (r c) -> p r c", c=W)
                o1 = pv1[:].rearrange("p (r c) -> p r c", c=W)
                rb = 4 * j
                for t, (di, dj) in enumerate(TAPS):
                    rhs0 = pbv0[:, rb + di:rb + di + 4, dj:dj + 128]
                    nc.tensor.matmul(o0, dv_sb[:, t * P0:(t + 1) * P0], rhs0,
                                     start=(t == 0), stop=(t == 8))
                for g in range(6):
                    di, dj = (g, 0) if g < 3 else (g - 3, 2)
                    rhs1 = pbf1[:, rb + di:rb + di + 4, dj:dj + 128]
                    nc.tensor.matmul(o1, dv1_sb[:, g * P0:(g + 1) * P0], rhs1,
                                     start=(g == 0), stop=(g == 5))
                vst0 = vsp.tile([P0, SUB], f16, tag="v0")
                vst1 = vsp.tile([P0, SUB], f16, tag="v1")
                nc.vector.tensor_copy(vst0[:], pv0[:])
                nc.vector.tensor_copy(vst1[:], pv1[:])
                return vst0, vst1

            och = {}

            def emit_final(j, vst0, vst1):
                """Fused final matmul: out = WfT-contraction @ v_dw.  All 4
                lhsT slices are uniform [128,128]; vst1 junk rows 64:128 meet
                the zero rows of the padded Wf."""
                c, s4 = j // 4, j % 4
                if s4 == 0:
                    och[c] = (osp.tile([P0, CHUNK], f32, tag="o0", name="och0"),
                              osp.tile([P1, CHUNK], f32, tag="o1", name="och1"))
                och0, och1 = och[c]
                po0 = psT0.tile([P0, SUB], f32, tag="t")
                po1 = psT1.tile([P0, SUB], f32, tag="t")
                nc.tensor.matmul(po0[:], wf_sb[:, 0:128], vst0[:],
                                 start=True, stop=False)
                nc.tensor.matmul(po1[:], wf_sb[:, 128:256], vst0[:],
                                 start=True, stop=False)
                nc.tensor.matmul(po0[:], wf_sb[:, 256:384], vst1[:],
                                 start=False, stop=True)
                nc.tensor.matmul(po1[:], wf_sb[:, 384:512], vst1[:],
                                 start=False, stop=True)
                nc.scalar.copy(och0[:, s4 * SUB:(s4 + 1) * SUB], po0[:])
                nc.vector.tensor_copy(och1[:, s4 * SUB:(s4 + 1) * SUB],
                                      po1[0:P1, :])
                if c == NCH - 1:
                    # last chunk: flush per subtile so the tail DMA is short
                    scs = slice(c * CHUNK + s4 * SUB, c * CHUNK + (s4 + 1) * SUB)
                    ssl = slice(s4 * SUB, (s4 + 1) * SUB)
                    nc.sync.dma_start(out=out[0:P0, scs], in_=och0[:, ssl])
                    nc.sync.dma_start(out=out[P0:C, scs], in_=och1[:, ssl])
                elif s4 == 3:
                    ocs = slice(c * CHUNK, (c + 1) * CHUNK)
                    nc.sync.dma_start(out=out[0:P0, ocs], in_=och0[:])
                    nc.sync.dma_start(out=out[P0:C, ocs], in_=och1[:])

            # Pre-emit the first two V chunks' pointwise and six tap groups
            # so the in-order PE queue has work while the softmax chain runs.
            xv = [dma_xchunk(0)]
            for s4 in range(4):
                pw_subtile(s4, xv[0], s4, wv_sb, s4 % 2 == 0)
            xv.append(dma_xchunk(1))
            xv.append(dma_xchunk(2))
            for s4 in range(4):
                pw_subtile(4 + s4, xv[1], s4, wv_sb, s4 % 2 == 0)
            xv.append(dma_xchunk(3))
            pend = [(j,) + emit_taps_v(j) for j in range(6)]

            # ---- row scales: rn = sqrt(temp) / ||qd_row|| ----
            # ACT Sqrt is low-precision; one Newton step on y=sqrt(ss).
            nc.vector.tensor_reduce(ssq[:, 0:1], ssq[:, 0:QSUB], Ax.X, Alu.add)
            nc.vector.tensor_reduce(ssq[0:P1, QSUB:QSUB + 1],
                                    ssq[0:P1, QSUB:2 * QSUB], Ax.X, Alu.add)
            for ss_ap, rn_ap, tq_ap in (
                (ssq[:, 0:1], rn[:, 0:1], tq_sb[:, 0:1]),
                (ssq[0:P1, QSUB:QSUB + 1], rn[0:P1, 1:2], tq_sb[0:P1, 1:2]),
            ):
                y = scr[0:ss_ap.shape[0], 0:1]
                yr = scr[0:ss_ap.shape[0], 1:2]
                nc.scalar.activation(y, ss_ap, Act.Sqrt)
                nc.vector.reciprocal(yr, y)                      # 1/y
                nc.vector.tensor_tensor(yr, yr, ss_ap, Alu.mult)  # ss/y
                nc.vector.tensor_tensor(y, y, yr, Alu.add)
                nc.vector.tensor_scalar_mul(y, y, 0.5)            # refined sqrt
                nc.vector.reciprocal(rn_ap, y)
                nc.vector.tensor_tensor(rn_ap, rn_ap, tq_ap, Alu.mult)

            # attn = diag(s) G diag(s): row scale by s_c, then elementwise
            # multiply by s_d replicated across partitions.
            nc.sync.dma_start(out=srow[0:1, 0:P0], in_=rn[:, 0:1])
            nc.sync.dma_start(out=srow[0:1, P0:C], in_=rn[0:P1, 1:2])
            nc.gpsimd.partition_broadcast(srow[:], srow[0:1, :])
            nc.vector.tensor_scalar_mul(g0a[:], g0a[:], rn[:, 0:1])
            nc.vector.tensor_scalar_mul(g1a[:], g1a[:], rn[0:P1, 1:2])
            nc.vector.tensor_tensor(g0a[:], g0a[:], srow[:], Alu.mult)
            nc.vector.tensor_tensor(g1a[:], g1a[:], srow[0:P1, :], Alu.mult)

            # ---- blockwise softmax over every 24-col block, then keep the
            # diagonal block per row via a precomputed mask ----
            gv0 = g0a[:].rearrange("p (h c) -> p h c", c=CHD)
            gv1 = g1a[:].rearrange("p (h c) -> p h c", c=CHD)
            # logits are cosine-similarities scaled by temperature (=1):
            # bounded, so exp is safe without the max-subtraction pass
            nc.scalar.activation(g0a[:], g0a[:], Act.Exp)
            nc.scalar.activation(g1a[:], g1a[:], Act.Exp)
            sm0 = stats[:, 2 * HEADS:3 * HEADS]
            sm1 = stats[0:P1, 3 * HEADS:4 * HEADS]
            nc.vector.tensor_reduce(sm0, gv0, Ax.X, Alu.add)
            nc.vector.tensor_reduce(sm1, gv1, Ax.X, Alu.add)
            nc.vector.reciprocal(sm0, sm0)
            nc.vector.reciprocal(sm1, sm1)
            nc.vector.tensor_tensor(gv0, gv0,
                                    sm0.unsqueeze(2).broadcast_to((P0, HEADS, CHD)),
                                    Alu.mult)
            nc.vector.tensor_tensor(gv1, gv1,
                                    sm1.unsqueeze(2).broadcast_to((P1, HEADS, CHD)),
                                    Alu.mult)
            nc.vector.tensor_tensor(g0a[:], g0a[:], mk0_sb[:], Alu.mult)
            nc.vector.tensor_tensor(g1a[:], g1a[:], mk1_sb[:], Alu.mult)
            nc.vector.tensor_copy(A0h[:], g0a[:])
            nc.vector.tensor_copy(A1h[:], g1a[:])
            if _dbg:
                nc.sync.dma_start(out=datt0, in_=g0a[:])
                nc.sync.dma_start(out=datt1, in_=g1a[:])

            # ---- WfT = blockdiag(A)-contraction with WpT (fp16) ----
            pwf0 = psT0.tile([P0, C], f32, tag="t")
            pwf1 = psT1.tile([P1, C], f32, tag="t")
            nc.tensor.matmul(pwf0[:], A0h[:, 0:P0], wp_sb[:, 0:192],
                             start=True, stop=False)
            nc.tensor.matmul(pwf0[:], A1h[:, 0:P0], wp_sb[0:P1, 192:384],
                             start=False, stop=True)
            nc.tensor.matmul(pwf1[:], A0h[:, P0:C], wp_sb[:, 0:192],
                             start=True, stop=False)
            nc.tensor.matmul(pwf1[:], A1h[:, P0:C], wp_sb[0:P1, 192:384],
                             start=False, stop=True)
            nc.scalar.copy(wf_sb[:, 0:192], pwf0[:])
            nc.scalar.copy(wf_sb[0:P1, 256:448], pwf1[:])
            if _dbg:
                nc.sync.dma_start(out=dwf, in_=wf_sb[:])

            # ================= V phase (remainder) =================
            for jv, v0p, v1p in pend:
                emit_final(jv, v0p, v1p)
            for s in range(7, NSUB + 1):
                if 8 <= s < NSUB:
                    pw_subtile(s, xv[s // 4], s % 4, wv_sb, s % 2 == 0)
                    if s % 4 == 3 and s // 4 + 2 < NCH:
                        xv.append(dma_xchunk(s // 4 + 2))
                j = s - 1
                v0p, v1p = emit_taps_v(j)
                emit_final(j, v0p, v1p)

    nc.compile()
    return nc


def _host_inputs(x, w_qkv, w_dw, w_proj, temperature):
    """Per-core input maps (host-side precompute of all weight transforms)."""
    f = np.float32
    W_q = w_qkv[0:C].astype(f)           # (192,192)
    W_v = w_qkv[2 * C:3 * C].astype(f)
    wq_d = w_dw[0:C, 0].reshape(C, 9).astype(f)        # (192,9) taps (di,dj)
    wv_d = w_dw[2 * C:3 * C, 0].reshape(C, 9).astype(f)

    def pack_T(Wm):
        """W^T channel-padded to 256 and packed as (128, 512):
        [:,0:256]=rows 0:128 of WT_pad (K-chunk0), [:,256:512]=rows 128:256
        (K-chunk1; rows 64:128 zero).  Output channels 192:256 duplicate
        128:192 so the ch1 image lands twice in the padded buffer, enabling
        paired depthwise taps."""
        WTp = np.zeros((256, 256), f)
        WTp[0:C, 0:C] = Wm.T.astype(f)
        WTp[:, 192:256] = WTp[:, 128:192]
        return np.concatenate([WTp[0:P0], WTp[P0:256]], axis=1)

    def pack_diag(wd, lo, n):
        """9 diagonal tap matrices, zero-padded to (128, 9*128)."""
        out = np.zeros((P0, 9 * P0), f)
        for t in range(9):
            np.fill_diagonal(out[0:n, t * P0:t * P0 + n], wd[lo:lo + n, t])
        return out

    def pack_diag1(wd):
        """ch1 tap matrices for the duplicated layout, (128, 6*128):
        g<3: paired taps (di,dj=0)+(di,dj=1) -- rows 0:64 scale the lower
        copy, rows 64:128 the col-shifted upper copy, both into out 0:64;
        g>=3: single tap (di,dj=2) on the lower copy only."""
        out = np.zeros((P0, 6 * P0), f)
        w = wd[P0:C].reshape(P1, 3, 3)
        for di in range(3):
            np.fill_diagonal(out[0:P1, di * P0:di * P0 + P1], w[:, di, 0])
            np.fill_diagonal(out[P1:P0, di * P0:di * P0 + P1], w[:, di, 1])
            g = 3 + di
            np.fill_diagonal(out[0:P1, g * P0:g * P0 + P1], w[:, di, 2])
        return out

    wp_pack = np.zeros((P0, 384), f)
    WpT = w_proj.T.astype(f)
    wp_pack[:, 0:192] = WpT[0:P0]
    wp_pack[0:P1, 192:384] = WpT[P0:C]

    tq = np.sqrt(np.repeat(temperature.reshape(HEADS).astype(f), CHD)).reshape(C, 1)
    eye = np.eye(P0, dtype=np.float16)

    heads = np.arange(C) // CHD
    mask_full = (heads[:, None] == heads[None, :]).astype(f)   # (192,192)

    shared = {
        "wq": pack_T(W_q), "wv": pack_T(W_v), "wp": wp_pack,
        "dq": pack_diag(wq_d, 0, P0), "dq1": pack_diag1(wq_d),
        "dv": pack_diag(wv_d, 0, P0), "dv1": pack_diag1(wv_d),
        "tq": tq, "eye": eye.astype(np.float16),
        "mk0": np.ascontiguousarray(mask_full[0:P0]),
        "mk1": np.ascontiguousarray(mask_full[P0:C]),
    }
    h = np.float16
    for k in ("wq", "wv", "wp", "dq", "dq1", "dv", "dv1"):
        shared[k] = shared[k].astype(h)
    maps = []
    for b in range(8):
        m = dict(shared)
        m["xb"] = np.ascontiguousarray(x[b].reshape(C, HW).astype(h))
        maps.append(m)
    return maps


def kernel(x, w_qkv, w_dw, w_proj, temperature, _trace=False, _iters=1):
    from concourse.bass_utils import run_bass_kernel_spmd
    if _iters not in _BUILT:
        _BUILT[_iters] = _build(_iters)
    nc = _BUILT[_iters]
    in_maps = _host_inputs(
        np.asarray(x), np.asarray(w_qkv), np.asarray(w_dw),
        np.asarray(w_proj), np.asarray(temperature))
    res = run_bass_kernel_spmd(nc, in_maps, list(range(8)), trace=_trace)
    outs = [res.results[i]["out"].reshape(C, H, W) for i in range(8)]
    y = np.stack(outs, axis=0).astype(np.float32)
    kernel.last_result = res
    return y


# revision 71
# speedup vs baseline: 1.0481x; 1.0067x over previous
"""Trainium2 Bass kernel for nn_Attention_45749991637079.

Reference computation (per batch b, C=192 channels, 128x128 image, 8 heads):
    qkv  = w_qkv @ x                       (1x1 conv; k-branch unused)
    q,v  = depthwise 3x3 (SAME) of the q/v channel blocks
    qd   = q[:, ::2, ::2]                  (64x64 downsample)
    attn = softmax(l2norm-rows(qd_h) gram * temp)   per head (24x24)
    out  = w_proj @ (attn @ v)             == (w_proj @ blockdiag(attn)) @ v

Sharding: data-parallel over batch; one batch per NeuronCore (8 cores).

Device algorithm per core (optimized for a warm, densely-fed PE):
  Q phase: whole-image pointwise conv into a zero-padded SBUF buffer
    (130x130 per chunk), 9-tap depthwise via diag-matmuls with stride-2
    views -> qd; per-128-col-block PE transposes -> gram accumulate.
  Softmax: row-norm scaling, blockwise softmax over all 24-col blocks,
    block-diagonal mask, Wf = blockdiag(A)^T-contraction with WpT.
  V phase (overlaps softmax): pointwise -> padded buffer -> 9-tap
    depthwise -> fp16 stage -> fused final matmul with Wf -> DMA out.
  All weight transposes / diag-tap matrices / masks precomputed on host.
"""

import numpy as np

C = 192
H = W = 128
HW = H * W
HEADS = 8
CHD = 24
P0, P1 = 128, 64          # channel partition chunks: 0:128 and 128:192
PB = 130                  # padded row width / height (1 + 128 + 1)
PBSZ = PB * PB            # padded image cols per chunk
SUB = 512                 # output subtile cols (4 image rows)
NSUB = HW // SUB          # 32
CHUNK = 2048              # x DMA chunk cols (16 image rows)
NCH = HW // CHUNK         # 8
QSUB = 8                  # qd subtiles (8 rows of 64 each)
TAPS = [(di, dj) for di in range(3) for dj in range(3)]

_BUILT = {}


def _build(iters=1):
    import concourse.mybir as mybir
    import concourse.tile as tile
    from concourse import bacc

    f32 = mybir.dt.float32
    f16 = mybir.dt.float16
    Alu = mybir.AluOpType
    Act = mybir.ActivationFunctionType
    Ax = mybir.AxisListType

    nc = bacc.Bacc(
        "TRN2", target_bir_lowering=False, debug=False,
        enable_asserts=False, num_devices=8,
    )

    # DRAM I/O (per-core shapes)
    xb = nc.dram_tensor("xb", (C, HW), f16, kind="ExternalInput").ap()
    wq = nc.dram_tensor("wq", (P0, 512), f16, kind="ExternalInput").ap()
    wv = nc.dram_tensor("wv", (P0, 512), f16, kind="ExternalInput").ap()
    wp = nc.dram_tensor("wp", (P0, 384), f16, kind="ExternalInput").ap()
    dq = nc.dram_tensor("dq", (P0, 9 * P0), f16, kind="ExternalInput").ap()
    dq1 = nc.dram_tensor("dq1", (P0, 6 * P0), f16, kind="ExternalInput").ap()
    dv = nc.dram_tensor("dv", (P0, 9 * P0), f16, kind="ExternalInput").ap()
    dv1 = nc.dram_tensor("dv1", (P0, 6 * P0), f16, kind="ExternalInput").ap()
    tq = nc.dram_tensor("tq", (C, 1), f32, kind="ExternalInput").ap()
    eye = nc.dram_tensor("eye", (P0, P0), f16, kind="ExternalInput").ap()
    mk0 = nc.dram_tensor("mk0", (P0, C), f32, kind="ExternalInput").ap()
    mk1 = nc.dram_tensor("mk1", (P1, C), f32, kind="ExternalInput").ap()
    out = nc.dram_tensor("out", (C, HW), f32, kind="ExternalOutput").ap()
    import os
    _dbg = os.environ.get("KDBG") == "1"
    if _dbg:
        dqd = nc.dram_tensor("dqd", (P0, 8192), f16, kind="ExternalOutput").ap()
        datt0 = nc.dram_tensor("datt0", (P0, C), f32, kind="ExternalOutput").ap()
        datt1 = nc.dram_tensor("datt1", (P1, C), f32, kind="ExternalOutput").ap()
        dwf = nc.dram_tensor("dwf", (P0, 512), f16, kind="ExternalOutput").ap()
        dgram = nc.dram_tensor("dgram", (P0, C), f32, kind="ExternalOutput").ap()
        dpb = nc.dram_tensor("dpb", (P0, 2 * PBSZ), f16, kind="ExternalOutput").ap()

    import contextlib

    with tile.TileContext(nc) as tc:
      with (tc.For_i(0, iters, 1) if iters > 1 else contextlib.nullcontext()):
        with (
            tc.tile_pool(name="const", bufs=1) as cp,
            tc.tile_pool(name="work", bufs=5) as wkp,
            tc.tile_pool(name="vst", bufs=12) as vsp,
            tc.tile_pool(name="ost", bufs=2) as osp,
            tc.tile_pool(name="psPW", bufs=4, space="PSUM") as psPW,
            tc.tile_pool(name="psT0", bufs=2, space="PSUM") as psT0,
            tc.tile_pool(name="psT1", bufs=2, space="PSUM") as psT1,
        ):
            # ---- constants ----
            # pw weights packed channel-padded to 256: [:,0:256]=WT_pad rows
            # 0:128 (K-chunk0), [:,256:512]=rows 128:256 (K-chunk1, rows 64:128
            # zero).  Uniform [128,128] lhsT slices keep the PE weight loads
            # pipelined (mixed tile shapes serialize LDWEIGHTS).
            wq_sb = cp.tile([P0, 512], f16)
            wv_sb = cp.tile([P0, 512], f16)
            wp_sb = cp.tile([P0, 384], f16)   # WpT rows 0:128 | rows 128:192
            dq_sb = cp.tile([P0, 9 * P0], f16)
            dq1_sb = cp.tile([P0, 6 * P0], f16)  # ch1 paired-tap matrices
            dv_sb = cp.tile([P0, 9 * P0], f16)
            dv1_sb = cp.tile([P0, 6 * P0], f16)
            tq_sb = cp.tile([P0, 2], f32)     # [:,0]=ch0..127, [0:64,1]=ch128..191
            eye_sb = cp.tile([P0, P0], f16)
            mk0_sb = cp.tile([P0, C], f32)    # blockdiag mask rows 0:128
            mk1_sb = cp.tile([P1, C], f32)    # rows 128:192
            pb = cp.tile([P0, 2 * PBSZ], f16)  # padded image: ch0 | ch1
            qd_sb = cp.tile([P0, 8192], f16)  # qd: [:,0:4096] | [0:64,4096:8192]
            g0a = cp.tile([P0, C], f32)       # gram accumulator rows 0:128
            g1a = cp.tile([P1, C], f32)       # rows 128:192
            srow = cp.tile([P0, C], f32)      # s_d broadcast to all partitions
            wf_sb = cp.tile([P0, 512], f16)   # WfT M-padded: K0 @0:256, K1 @256:512
            A0h = cp.tile([P0, C], f16)       # masked softmax attn (fp16)
            A1h = cp.tile([P1, C], f16)
            ssq = cp.tile([P0, 2 * QSUB], f32)  # row sum-of-squares per subtile
            stats = cp.tile([P0, 4 * HEADS], f32)  # softmax stats
            rn = cp.tile([P0, 2], f32)        # 1/||q|| * sqrt(temp)
            scr = cp.tile([P0, SUB], f32)     # scratch for sumsq STT

            pbv0 = pb[:, 0:PBSZ].rearrange("p (r c) -> p r c", c=PB)
            pbv1 = pb[0:P1, PBSZ:2 * PBSZ].rearrange("p (r c) -> p r c", c=PB)
            pbf1 = pb[:, PBSZ:2 * PBSZ].rearrange("p (r c) -> p r c", c=PB)

            # manual 3-slot x ring: ch1 junk partitions zeroed once (on DVE,
            # which is idle at the start) so the channel-padded pw matmuls
            # never stream NaN bit patterns.
            xslots = []
            for _i in range(3):
                xs_t = cp.tile([P0, 2 * CHUNK], f16, name=f"xslot{_i}")
                xslots.append(xs_t)
            for xs in xslots:
                nc.vector.memset(xs[P1:P0, CHUNK:2 * CHUNK], 0.0)

            def dma_xchunk(c):
                xt = xslots[c % 3]
                cs = slice(c * CHUNK, (c + 1) * CHUNK)
                nc.sync.dma_start(out=xt[:, 0:CHUNK], in_=xb[0:P0, cs])
                nc.sync.dma_start(out=xt[0:P1, CHUNK:2 * CHUNK], in_=xb[P0:C, cs])
                return xt

            # HAM warm-up: dependency-free matmuls into dead PSUM slots keep
            # the PE busy through its 3.4us activity window while the first
            # DMAs land, so the real stream starts at full clock.
            for wu in range(16):
                pwu = (psT0 if wu % 2 == 0 else psT1).tile(
                    [P0, 256], f32, tag="t", name="warm")
                nc.tensor.matmul(pwu[:], pb[:, 0:P0], pb[:, 1024:1280],
                                 start=True, stop=True)

            # the first pw matmuls need only wq + x chunk 0, and each
            # dma_start costs ~600ns of Sync issue time: those go first.
            nc.sync.dma_start(out=wq_sb[:], in_=wq[:])
            xq = [dma_xchunk(0)]
            xq.append(dma_xchunk(1))
            nc.sync.dma_start(out=dq_sb[:], in_=dq[:])
            nc.sync.dma_start(out=dq1_sb[:], in_=dq1[:])
            nc.sync.dma_start(out=wv_sb[:], in_=wv[:])
            nc.sync.dma_start(out=wp_sb[:, 0:192], in_=wp[:, 0:192])
            nc.sync.dma_start(out=wp_sb[0:P1, 192:384], in_=wp[0:P1, 192:384])
            nc.sync.dma_start(out=dv_sb[:], in_=dv[:])
            nc.sync.dma_start(out=dv1_sb[:], in_=dv1[:])
            nc.sync.dma_start(out=tq_sb[:, 0:1], in_=tq[0:P0, :])
            nc.sync.dma_start(out=tq_sb[0:P1, 1:2], in_=tq[P0:C, :])
            nc.sync.dma_start(out=eye_sb[:], in_=eye[:])
            nc.sync.dma_start(out=mk0_sb[:], in_=mk0[:])
            nc.sync.dma_start(out=mk1_sb[:], in_=mk1[:])

            # zero the pad rows/cols once; the ch1 region is padded across
            # all 128 partitions (its interior rows 64:128 are rewritten with
            # real zeros by every full-partition pw copy).
            nc.gpsimd.memset(pbv0[:, :, 0:1], 0.0)
            nc.gpsimd.memset(pbv0[:, :, PB - 1:PB], 0.0)
            nc.gpsimd.memset(pbv0[:, 0, :], 0.0)
            nc.gpsimd.memset(pbv0[:, PB - 1, :], 0.0)
            nc.gpsimd.memset(pbf1[:, :, 0:1], 0.0)
            nc.gpsimd.memset(pbf1[:, :, PB - 1:PB], 0.0)
            nc.gpsimd.memset(pbf1[:, 0, :], 0.0)
            nc.gpsimd.memset(pbf1[:, PB - 1, :], 0.0)
            # upper duplicate stores image col m at buffer col m; its cols
            # 128..129 are never written and must stay zero
            nc.gpsimd.memset(pbf1[P1:P0, :, P0:PB - 1], 0.0)

            nc.gpsimd.memset(g0a[:], 0.0)
            nc.gpsimd.memset(g1a[:], 0.0)
            nc.gpsimd.memset(wf_sb[:], 0.0)   # zero rows/cols of the padding

            def pw_subtile(s, xt, s4, w_sb, act_first):
                """Pointwise conv of image rows 4s..4s+3 into padded buffer.
                All 4 matmuls use uniform [128,128] lhsT (channel-padded);
                junk rows 64:128 of the ch1 rhs meet zero weight rows."""
                xr0 = xt[:, s4 * SUB:(s4 + 1) * SUB]
                xr1 = xt[:, CHUNK + s4 * SUB:CHUNK + (s4 + 1) * SUB]
                pp0 = psPW.tile([P0, SUB], f32, tag="pw")
                pp1 = psPW.tile([P0, SUB], f32, tag="pw")
                nc.tensor.matmul(pp0[:], w_sb[:, 0:128], xr0, start=True, stop=False)
                nc.tensor.matmul(pp1[:], w_sb[:, 128:256], xr0, start=True, stop=False)
                nc.tensor.matmul(pp0[:], w_sb[:, 256:384], xr1, start=False, stop=True)
                nc.tensor.matmul(pp1[:], w_sb[:, 384:512], xr1, start=False, stop=True)
                r0 = 4 * s + 1   # buffer row of image row 4s
                d0 = pbv0[:, r0:r0 + 4, 1:129]
                # ch1 lands twice: lower copy at the standard +1 offset and
                # the duplicated upper partitions shifted one column left,
                # which lets two horizontal taps share one matmul.
                d1a = pbf1[0:P1, r0:r0 + 4, 1:129]
                d1b = pbf1[P1:P0, r0:r0 + 4, 0:128]
                v0 = pp0[:].rearrange("p (r c) -> p r c", c=W)
                v1 = pp1[:].rearrange("p (r c) -> p r c", c=W)
                if act_first:
                    nc.scalar.copy(d0, v0)
                    nc.vector.tensor_copy(d1a, v1[0:P1])
                    nc.vector.tensor_copy(d1b, v1[P1:P0])
                else:
                    nc.vector.tensor_copy(d0, v0)
                    nc.scalar.copy(d1a, v1[0:P1])
                    nc.scalar.copy(d1b, v1[P1:P0])

            # ================= Q phase =================
            # pointwise runs one chunk ahead of the taps, and the gram of
            # chunk c runs one iteration late so its transposes never wait
            # on the freshly-written qd copies.
            for s4 in range(4):
                pw_subtile(s4, xq[0], s4, wq_sb, s4 % 2 == 0)

            def emit_gram(k):
                """Gram contribution of qd subtile k's 4 128-col blocks
                (matmuls batched by lhsT shape to keep weight loads
                pipelined)."""
                pg0 = psT0.tile([P0, C], f32, tag="t")
                pg1 = psT1.tile([P1, C], f32, tag="t")
                qdTs = []
                for b4 in range(4):
                    kcol = k * SUB + b4 * P0
                    pt0 = psPW.tile([P0, P0], f16, tag="pw")
                    pt1 = psPW.tile([P0, P1], f16, tag="pw")
                    nc.tensor.transpose(pt0[:], qd_sb[:, kcol:kcol + P0], eye_sb[:])
                    nc.tensor.transpose(pt1[:], qd_sb[0:P1, 4096 + kcol:4096 + kcol + P0],
                                        eye_sb[0:P1, 0:P1])
                    qdT = wkp.tile([P0, C], f16, tag="qdT")
                    nc.vector.tensor_copy(qdT[:, 0:P0], pt0[:])
                    nc.vector.tensor_copy(qdT[:, P0:C], pt1[:])
                    qdTs.append(qdT)
                # the masked softmax only reads the diagonal head blocks:
                # rows 0:128 need cols 0:144 (heads 0-5), rows 128:192 need
                # cols 120:192 (heads 5-7); the rest of g0a/g1a stays zero.
                for b4 in range(4):
                    nc.tensor.matmul(pg0[:, 0:144], qdTs[b4][:, 0:P0],
                                     qdTs[b4][:, 0:144],
                                     start=(b4 == 0), stop=(b4 == 3))
                for b4 in range(4):
                    nc.tensor.matmul(pg1[:, 0:72], qdTs[b4][:, P0:C],
                                     qdTs[b4][:, 120:192],
                                     start=(b4 == 0), stop=(b4 == 3))
                nc.vector.tensor_tensor(g0a[:, 0:144], g0a[:, 0:144],
                                        pg0[:, 0:144], Alu.add)
                nc.vector.tensor_tensor(g1a[:, 120:192], g1a[:, 120:192],
                                        pg1[:, 0:72], Alu.add)

            for c in range(NCH):
                if c + 1 < NCH:
                    for s4 in range(4):
                        pw_subtile(4 * (c + 1) + s4, xq[c + 1], s4, wq_sb,
                                   s4 % 2 == 0)
                    if c + 2 < NCH:
                        xq.append(dma_xchunk(c + 2))
                # taps for qd subtile k=c (qd rows 8c..8c+7); ch1 uses the
                # zero-padded [128,128] diagonals on the full-partition view
                k = c
                pq0 = psT0.tile([P0, SUB], f32, tag="t")
                pq1 = psT1.tile([P0, SUB], f32, tag="t")
                o0 = pq0[:].rearrange("p (r c) -> p r c", c=64)
                o1 = pq1[:].rearrange("p (r c) -> p r c", c=64)
                rb = 16 * k  # buffer row of qd row 8k input base (2i, i=8k)
                for t, (di, dj) in enumerate(TAPS):
                    rhs0 = pbv0[:, rb + di:rb + di + 16:2, dj:dj + 128:2]
                    nc.tensor.matmul(o0, dq_sb[:, t * P0:(t + 1) * P0], rhs0,
                                     start=(t == 0), stop=(t == 8))
                for g in range(6):
                    di, dj = (g, 0) if g < 3 else (g - 3, 2)
                    rhs1 = pbf1[:, rb + di:rb + di + 16:2, dj:dj + 128:2]
                    nc.tensor.matmul(o1, dq1_sb[:, g * P0:(g + 1) * P0], rhs1,
                                     start=(g == 0), stop=(g == 5))
                nc.scalar.activation(scr[:], pq0[:], Act.Square,
                                     accum_out=ssq[:, k:k + 1])
                nc.scalar.activation(scr[0:P1, :], pq1[0:P1, :], Act.Square,
                                     accum_out=ssq[0:P1, QSUB + k:QSUB + k + 1])
                nc.vector.tensor_copy(qd_sb[:, k * SUB:(k + 1) * SUB], pq0[:])
                nc.vector.tensor_copy(qd_sb[0:P1, 4096 + k * SUB:4096 + (k + 1) * SUB],
                                      pq1[0:P1, :])
                if c >= 1:
                    emit_gram(c - 1)
            emit_gram(NCH - 1)

            if _dbg:
                nc.sync.dma_start(out=dqd, in_=qd_sb[:])
                nc.sync.dma_start(out=dgram, in_=g0a[:])
                nc.sync.dma_start(out=dpb, in_=pb[:])
            # ================= V phase helpers =================
            # taps for subtile j need pad rows 4j..4j+5; row 4j+5 is written
            # by pw subtile j+1, so taps lag the pointwise by one subtile.
            def emit_taps_v(j):
                pv0 = psT0.tile([P0, SUB], f32, tag="t")
                pv1 = psT1.tile([P0, SUB], f32, tag="t")
                o0 = pv0[:].rearrange("p (r c) -> p r c", c=W)
                o1 = pv1[:].rearrange("p (r c) -> p r c", c=W)
                rb = 4 * j
                for t, (di, dj) in enumerate(TAPS):
                    rhs0 = pbv0[:, rb + di:rb + di + 4, dj:dj + 128]
                    nc.tensor.matmul(o0, dv_sb[:, t * P0:(t + 1) * P0], rhs0,
                                     start=(t == 0), stop=(t == 8))
                for g in range(6):
                    di, dj = (g, 0) if g < 3 else (g - 3, 2)
                    rhs1 = pbf1[:, rb + di:rb + di + 4, dj:dj + 128]
                    nc.tensor.matmul(o1, dv1_sb[:, g * P0:(g + 1) * P0], rhs1,
                                     start=(g == 0), stop=(g == 5))
                vst0 = vsp.tile([P0, SUB], f16, tag="v0")
                vst1 = vsp.tile([P0, SUB], f16, tag="v1")
                nc.vector.tensor_copy(vst0[:], pv0[:])
                nc.vector.tensor_copy(vst1[:], pv1[:])
                return vst0, vst1

            och = {}

            def emit_final(j, vst0, vst1):
                """Fused final matmul: out = WfT-contraction @ v_dw.  All 4
                lhsT slices are uniform [128,128]; vst1 junk rows 64:128 meet
                the zero rows of the padded Wf."""
                c, s4 = j // 4, j % 4
                if s4 == 0:
                    och[c] = (osp.tile([P0, CHUNK], f32, tag="o0", name="och0"),
                              osp.tile([P1, CHUNK], f32, tag="o1", name="och1"))
                och0, och1 = och[c]
                po0 = psT0.tile([P0, SUB], f32, tag="t")
                po1 = psT1.tile([P0, SUB], f32, tag="t")
                nc.tensor.matmul(po0[:], wf_sb[:, 0:128], vst0[:],
                                 start=True, stop=False)
                nc.tensor.matmul(po1[:], wf_sb[:, 128:256], vst0[:],
                                 start=True, stop=False)
                nc.tensor.matmul(po0[:], wf_sb[:, 256:384], vst1[:],
                                 start=False, stop=True)
                nc.tensor.matmul(po1[:], wf_sb[:, 384:512], vst1[:],
                                 start=False, stop=True)
                nc.scalar.copy(och0[:, s4 * SUB:(s4 + 1) * SUB], po0[:])
                nc.vector.tensor_copy(och1[:, s4 * SUB:(s4 + 1) * SUB],
                                      po1[0:P1, :])
                if c == NCH - 1:
                    # last chunk: flush per subtile so the tail DMA is short
                    scs = slice(c * CHUNK + s4 * SUB, c * CHUNK + (s4 + 1) * SUB)
                    ssl = slice(s4 * SUB, (s4 + 1) * SUB)
                    nc.sync.dma_start(out=out[0:P0, scs], in_=och0[:, ssl])
                    nc.sync.dma_start(out=out[P0:C, scs], in_=och1[:, ssl])
                elif s4 == 3:
                    ocs = slice(c * CHUNK, (c + 1) * CHUNK)
                    nc.sync.dma_start(out=out[0:P0, ocs], in_=och0[:])
                    nc.sync.dma_start(out=out[P0:C, ocs], in_=och1[:])

            # Pre-emit the first two V chunks' pointwise and six tap groups
            # so the in-order PE queue has work while the softmax chain runs.
            xv = [dma_xchunk(0)]
            for s4 in range(4):
                pw_subtile(s4, xv[0], s4, wv_sb, s4 % 2 == 0)
            xv.append(dma_xchunk(1))
            xv.append(dma_xchunk(2))
            for s4 in range(4):
                pw_subtile(4 + s4, xv[1], s4, wv_sb, s4 % 2 == 0)
            xv.append(dma_xchunk(3))
            pend = [(j,) + emit_taps_v(j) for j in range(6)]

            # ---- row scales: rn = sqrt(temp) / ||qd_row|| ----
            # ACT Sqrt is low-precision; one Newton step on y=sqrt(ss).
            nc.vector.tensor_reduce(ssq[:, 0:1], ssq[:, 0:QSUB], Ax.X, Alu.add)
            nc.vector.tensor_reduce(ssq[0:P1, QSUB:QSUB + 1],
                                    ssq[0:P1, QSUB:2 * QSUB], Ax.X, Alu.add)
            for ss_ap, rn_ap, tq_ap in (
                (ssq[:, 0:1], rn[:, 0:1], tq_sb[:, 0:1]),
                (ssq[0:P1, QSUB:QSUB + 1], rn[0:P1, 1:2], tq_sb[0:P1, 1:2]),
            ):
                y = scr[0:ss_ap.shape[0], 0:1]
                yr = scr[0:ss_ap.shape[0], 1:2]
                nc.scalar.activation(y, ss_ap, Act.Sqrt)
                nc.vector.reciprocal(yr, y)                      # 1/y
                nc.vector.tensor_tensor(yr, yr, ss_ap, Alu.mult)  # ss/y
                nc.vector.tensor_tensor(y, y, yr, Alu.add)
                nc.vector.tensor_scalar_mul(y, y, 0.5)            # refined sqrt
                nc.vector.reciprocal(rn_ap, y)
                nc.vector.tensor_tensor(rn_ap, rn_ap, tq_ap, Alu.mult)

            # attn = diag(s) G diag(s): row scale by s_c, then elementwise
            # multiply by s_d replicated across partitions.
            nc.sync.dma_start(out=srow[0:1, 0:P0], in_=rn[:, 0:1])
            nc.sync.dma_start(out=srow[0:1, P0:C], in_=rn[0:P1, 1:2])
            nc.gpsimd.partition_broadcast(srow[:], srow[0:1, :])
            nc.vector.tensor_scalar_mul(g0a[:], g0a[:], rn[:, 0:1])
            nc.vector.tensor_scalar_mul(g1a[:], g1a[:], rn[0:P1, 1:2])
            nc.vector.tensor_tensor(g0a[:], g0a[:], srow[:], Alu.mult)
            nc.vector.tensor_tensor(g1a[:], g1a[:], srow[0:P1, :], Alu.mult)

            # ---- blockwise softmax over every 24-col block, then keep the
            # diagonal block per row via a precomputed mask ----
            gv0 = g0a[:].rearrange("p (h c) -> p h c", c=CHD)
            gv1 = g1a[:].rearrange("p (h c) -> p h c", c=CHD)
            # logits are cosine-similarities scaled by temperature (=1):
            # bounded, so exp is safe without the max-subtraction pass
            nc.scalar.activation(g0a[:], g0a[:], Act.Exp)
            nc.scalar.activation(g1a[:], g1a[:], Act.Exp)
            sm0 = stats[:, 2 * HEADS:3 * HEADS]
            sm1 = stats[0:P1, 3 * HEADS:4 * HEADS]
            nc.vector.tensor_reduce(sm0, gv0, Ax.X, Alu.add)
            nc.vector.tensor_reduce(sm1, gv1, Ax.X, Alu.add)
            nc.vector.reciprocal(sm0, sm0)
            nc.vector.reciprocal(sm1, sm1)
            nc.vector.tensor_tensor(gv0, gv0,
                                    sm0.unsqueeze(2).broadcast_to((P0, HEADS, CHD)),
                                    Alu.mult)
            nc.vector.tensor_tensor(gv1, gv1,
                                    sm1.unsqueeze(2).broadcast_to((P1, HEADS, CHD)),
                                    Alu.mult)
            nc.vector.tensor_tensor(g0a[:], g0a[:], mk0_sb[:], Alu.mult)
            nc.vector.tensor_tensor(g1a[:], g1a[:], mk1_sb[:], Alu.mult)
            nc.vector.tensor_copy(A0h[:], g0a[:])
            nc.vector.tensor_copy(A1h[:], g1a[:])
            if _dbg:
                nc.sync.dma_start(out=datt0, in_=g0a[:])
                nc.sync.dma_start(out=datt1, in_=g1a[:])

            # ---- WfT = blockdiag(A)-contraction with WpT (fp16) ----
            pwf0 = psT0.tile([P0, C], f32, tag="t")
            pwf1 = psT1.tile([P1, C], f32, tag="t")
            nc.tensor.matmul(pwf0[:], A0h[:, 0:P0], wp_sb[:, 0:192],
                             start=True, stop=False)
            nc.tensor.matmul(pwf0[:], A1h[:, 0:P0], wp_sb[0:P1, 192:384],
                             start=False, stop=True)
            nc.tensor.matmul(pwf1[:], A0h[:, P0:C], wp_sb[:, 0:192],
                             start=True, stop=False)
            nc.tensor.matmul(pwf1[:], A1h[:, P0:C], wp_sb[0:P1, 192:384],
                             start=False, stop=True)
            nc.scalar.copy(wf_sb[:, 0:192], pwf0[:])
            nc.scalar.copy(wf_sb[0:P1, 256:448], pwf1[:])
            if _dbg:
                nc.sync.dma_start(out=dwf, in_=wf_sb[:])

            # ================= V phase (remainder) =================
            for jv, v0p, v1p in pend:
                emit_final(jv, v0p, v1p)
            for s in range(7, NSUB + 1):
                if 8 <= s < NSUB:
                    pw_subtile(s, xv[s // 4], s % 4, wv_sb, s % 2 == 0)
                    if s % 4 == 3 and s // 4 + 2 < NCH:
                        xv.append(dma_xchunk(s // 4 + 2))
                j = s - 1
                v0p, v1p = emit_taps_v(j)
                emit_final(j, v0p, v1p)

    nc.compile()
    return nc


def _host_inputs(x, w_qkv, w_dw, w_proj, temperature):
    """Per-core input maps (host-side precompute of all weight transforms)."""
    f = np.float32
    W_q = w_qkv[0:C].astype(f)           # (192,192)
    W_v = w_qkv[2 * C:3 * C].astype(f)
    wq_d = w_dw[0:C, 0].reshape(C, 9).astype(f)        # (192,9) taps (di,dj)
    wv_d = w_dw[2 * C:3 * C, 0].reshape(C, 9).astype(f)

    def pack_T(Wm):
        """W^T channel-padded to 256 and packed as (128, 512):
        [:,0:256]=rows 0:128 of WT_pad (K-chunk0), [:,256:512]=rows 128:256
        (K-chunk1; rows 64:128 zero).  Output channels 192:256 duplicate
        128:192 so the ch1 image lands twice in the padded buffer, enabling
        paired depthwise taps."""
        WTp = np.zeros((256, 256), f)
        WTp[0:C, 0:C] = Wm.T.astype(f)
        WTp[:, 192:256] = WTp[:, 128:192]
        return np.concatenate([WTp[0:P0], WTp[P0:256]], axis=1)

    def pack_diag(wd, lo, n):
        """9 diagonal tap matrices, zero-padded to (128, 9*128)."""
        out = np.zeros((P0, 9 * P0), f)
        for t in range(9):
            np.fill_diagonal(out[0:n, t * P0:t * P0 + n], wd[lo:lo + n, t])
        return out

    def pack_diag1(wd):
        """ch1 tap matrices for the duplicated layout, (128, 6*128):
        g<3: paired taps (di,dj=0)+(di,dj=1) -- rows 0:64 scale the lower
        copy, rows 64:128 the col-shifted upper copy, both into out 0:64;
        g>=3: single tap (di,dj=2) on the lower copy only."""
        out = np.zeros((P0, 6 * P0), f)
        w = wd[P0:C].reshape(P1, 3, 3)
        for di in range(3):
            np.fill_diagonal(out[0:P1, di * P0:di * P0 + P1], w[:, di, 0])
            np.fill_diagonal(out[P1:P0, di * P0:di * P0 + P1], w[:, di, 1])
            g = 3 + di
            np.fill_diagonal(out[0:P1, g * P0:g * P0 + P1], w[:, di, 2])
        return out

    wp_pack = np.zeros((P0, 384), f)
    WpT = w_proj.T.astype(f)
    wp_pack[:, 0:192] = WpT[0:P0]
    wp_pack[0:P1, 192:384] = WpT[P0:C]

    tq = np.sqrt(np.repeat(temperature.reshape(HEADS).astype(f), CHD)).reshape(C, 1)
    eye = np.eye(P0, dtype=np.float16)

    heads = np.arange(C) // CHD
    mask_full = (heads[:, None] == heads[None, :]).astype(f)   # (192,192)

    shared = {
        "wq": pack_T(W_q), "wv": pack_T(W_v), "wp": wp_pack,
        "dq": pack_diag(wq_d, 0, P0), "dq1": pack_diag1(wq_d),
        "dv": pack_diag(wv_d, 0, P0), "dv1": pack_diag1(wv_d),
        "tq": tq, "eye": eye.astype(np.float16),
        "mk0": np.ascontiguousarray(mask_full[0:P0]),
        "mk1": np.ascontiguousarray(mask_full[P0:C]),
    }
    h = np.float16
    for k in ("wq", "wv", "wp", "dq", "dq1", "dv", "dv1"):
        shared[k] = shared[k].astype(h)
    maps = []
    for b in range(8):
        m = dict(shared)
        m["xb"] = np.ascontiguousarray(x[b].reshape(C, HW).astype(h))
        maps.append(m)
    return maps


def kernel(x, w_qkv, w_dw, w_proj, temperature, _trace=False, _iters=1):
    from concourse.bass_utils import run_bass_kernel_spmd
    if _iters not in _BUILT:
        _BUILT[_iters] = _build(_iters)
    nc = _BUILT[_iters]
    in_maps = _host_inputs(
        np.asarray(x), np.asarray(w_qkv), np.asarray(w_dw),
        np.asarray(w_proj), np.asarray(temperature))
    res = run_bass_kernel_spmd(nc, in_maps, list(range(8)), trace=_trace)
    outs = [res.results[i]["out"].reshape(C, H, W) for i in range(8)]
    y = np.stack(outs, axis=0).astype(np.float32)
    kernel.last_result = res
    return y
